# revision 1
# baseline (speedup 1.0000x reference)
"""Trainium2 Bass kernel for nn_CP_Attention_Action (dense transformer block with
CP-factored low-rank corrections).

Data-parallel over batch B=8 -> one batch per NeuronCore, no collectives.
Per core, feature-on-partition (transposed) layout:
  qkT (o,n) via stationary W tiles (CP branch fused into the PSUM group);
  v in natural (m,d) layout so it is the attn@v stationary operand, with a
  ones column appended so the softmax denominator falls out of the same
  matmul; logits via 64x128 row-tiled matmul pairs (two heads concurrently);
  exp on ScalarE with fused mask-bias + scale; 1/denom via K=1 bf16 matmul
  broadcast of the raw denominator + reciprocal_approx_fast on DVE; proj
  (+CP branch) in transposed layout; host transposes (o,n) -> (n,o).
"""

import numpy as np
import ml_dtypes

import concourse.bass as bass
from concourse import bacc
import concourse.mybir as mybir
import concourse.tile as tile
from concourse.bass_utils import run_bass_kernel_spmd

B, N, DIM = 8, 1024, 512
H, D = 8, 64
R = 64
SCALE = D ** -0.5
NCORES = 8
NC_CH = 2          # n chunks of 512
NT = N // 128      # 8 m-tiles
CT = DIM // 128    # 4 c-tiles
OT = 8             # q,k o-tiles
F32 = mybir.dt.float32
BF = mybir.dt.bfloat16
AF = mybir.ActivationFunctionType
bf16 = ml_dtypes.bfloat16

_CACHE = {}


def _build():
    nc = bacc.Bacc()

    xT_e = nc.declare_dram_parameter("xT", [DIM, N], BF, isOutput=False)
    wqkvT_e = nc.declare_dram_parameter("wqkvT", [DIM, 3 * DIM], BF, isOutput=False)
    cpuwT_e = nc.declare_dram_parameter("cpuwT", [DIM, R], BF, isOutput=False)
    cpvwT_e = nc.declare_dram_parameter("cpvwT", [R, DIM], BF, isOutput=False)
    cpvw65_e = nc.declare_dram_parameter("cpvw65", [R + 1, DIM], BF, isOutput=False)
    cpcrt_e = nc.declare_dram_parameter("cpcrt", [R, R * R], BF, isOutput=False)
    cpatt_e = nc.declare_dram_parameter("cpatt", [R, 4], BF, isOutput=False)
    wprojT_e = nc.declare_dram_parameter("wprojT", [DIM, DIM], BF, isOutput=False)
    fcon_e = nc.declare_dram_parameter("fcon", [128, 32], F32, isOutput=False)
    out_e = nc.declare_dram_parameter("out", [DIM, N], F32, isOutput=True)

    fdram = nc.dram_tensor("fdram", [4, R * R], BF)

    # fcon layout (f32 columns): 0:4 cpvb | 4:8 pbias | 8:16 maskb | 16 cpub(rows 0:64)
    with tile.TileContext(nc) as tc:
        with tc.tile_pool(name="consts", bufs=1) as consts, \
             tc.tile_pool(name="qkpool", bufs=1) as qkpool, \
             tc.tile_pool(name="stpool", bufs=4) as stpool, \
             tc.tile_pool(name="work", bufs=1) as work, \
             tc.tile_pool(name="dnpool", bufs=4) as dnpool, \
             tc.tile_pool(name="outp", bufs=1) as outp, \
             tc.tile_pool(name="popool", bufs=2) as popool:

            # ---------- constants / inputs (F-path + x first, weights split) ----------
            cbf = consts.tile([128, 1536], BF)
            nc.sync.dma_start(cbf[0:R, 1024:1028], cpatt_e[:, :])
            cpcrt = consts.tile([R, R * R], BF)
            nc.sync.dma_start(cpcrt[:], cpcrt_e[:, :])
            xT = consts.tile([128, CT, N], BF)
            nc.sync.dma_start(xT[:], xT_e[:, :].rearrange("(ct p) n -> p ct n", p=128))
            cpuw = consts.tile([128, CT, R], BF)
            nc.sync.dma_start(cpuw[:], cpuwT_e[:, :].rearrange("(ct p) r -> p ct r", p=128))
            fcon = consts.tile([128, 32], F32)
            nc.sync.dma_start(fcon[:], fcon_e[:, :])
            wqkv = consts.tile([128, CT, 3 * DIM], BF)
            nc.gpsimd.dma_start(wqkv[:], wqkvT_e[:, :].rearrange("(ct p) o -> p ct o", p=128))
            nc.gpsimd.dma_start(cbf[0:R + 1, 0:DIM], cpvw65_e[:, :])
            nc.gpsimd.dma_start(cbf[0:R, DIM:2 * DIM], cpvwT_e[:, :])
            wproj = consts.tile([128, CT, DIM], BF)
            nc.gpsimd.dma_start(wproj[:], wprojT_e[:, :].rearrange("(ct p) o -> p ct o", p=128))
            ones_bf = consts.tile([1, R], BF)
            nc.vector.memset(ones_bf[:], 1.0)

            def cpvw65(): return cbf[0:R + 1, 0:DIM]
            def cpvw(dt_): return cbf[0:R, DIM + dt_ * 128:DIM + (dt_ + 1) * 128]
            def cpatt(): return cbf[0:R, 1024:1028]
            def F_sb(f): return cbf[0:R, 1028 + f * R:1028 + (f + 1) * R]
            def cpvb(dt_): return fcon[:, dt_:dt_ + 1]
            def pbias(dt_): return fcon[:, 4 + dt_:5 + dt_]
            def maskb(mt): return fcon[:, 8 + mt:9 + mt]
            def cpub(): return fcon[0:R, 16:17]

            # ---------- F = CP_C x CP_attention (transient pools) ----------
            with tc.tile_pool(name="fsb", bufs=1) as fsb, \
                 tc.tile_pool(name="psf", bufs=3, space="PSUM") as psf:
                fflat = fsb.tile([4, R * R], BF)
                for ch in range(8):
                    csl = slice(ch * 512, (ch + 1) * 512)
                    fpc = psf.tile([4, 512], F32, tag="f", name=f"fp{ch}")
                    nc.tensor.matmul(fpc[:, :], lhsT=cpatt(), rhs=cpcrt[0:R, csl],
                                     start=True, stop=True)
                    if ch % 2 == 0:
                        nc.vector.tensor_copy(fflat[0:4, csl], fpc[:, :])
                    else:
                        nc.scalar.copy(fflat[0:4, csl], fpc[:, :])
                nc.sync.dma_start(fdram[:, 0:2048], fflat[0:4, 0:2048])
                nc.sync.dma_start(fdram[:, 2048:4096], fflat[0:4, 2048:4096])
                for f in range(4):
                    nc.sync.dma_start(cbf[0:R, 1028 + f * R:1028 + (f + 1) * R],
                                      fdram[f].rearrange("(r s) -> r s", s=R))

            outT = outp.tile([128, CT, N], BF)
            uu = work.tile([128, 2, N], BF)   # rows 0:64 -> [0]=u, [1]=u2
            tt = work.tile([128, 3, N], BF)   # rows 0:65; [2] has ones row for v
            v_sb = work.tile([128, NT, H, D + 1], BF)
            qk = qkpool.tile([128, OT, N], BF)

            # ================= phase 1: u/t, v, first qk pair =================
            with tc.tile_pool(name="ps_pre", bufs=4, space="PSUM") as ps_pre, \
                 tc.tile_pool(name="ps_sm", bufs=2, space="PSUM") as ps_sm:

                # u = CP_U(x)
                for ncx in range(NC_CH):
                    nsl = slice(ncx * 512, (ncx + 1) * 512)
                    up = ps_sm.tile([R, 512], F32, tag="sm", name=f"up{ncx}")
                    for ct in range(CT):
                        nc.tensor.matmul(up[:, :], lhsT=cpuw[:, ct, :], rhs=xT[:, ct, nsl],
                                         start=(ct == 0), stop=(ct == CT - 1))
                    nc.vector.tensor_scalar_add(uu[0:R, 0, nsl], up[:, :], cpub())

                # qk main matmuls for the first pair (t-independent)
                qk04 = {}
                for ot in (0, 4):
                    qps = [ps_pre.tile([128, 512], F32, tag="big", name=f"qp{ot}_{i}")
                           for i in range(NC_CH)]
                    qk04[ot] = qps
                    for ct in range(CT):
                        for ncx in range(NC_CH):
                            nc.tensor.matmul(qps[ncx][:, :],
                                             lhsT=wqkv[:, ct, ot * 128:(ot + 1) * 128],
                                             rhs=xT[:, ct, ncx * 512:(ncx + 1) * 512],
                                             start=(ct == 0), stop=False)

                # t_f = F_f.T @ uT
                for f in range(3):
                    for ncx in range(NC_CH):
                        nsl = slice(ncx * 512, (ncx + 1) * 512)
                        tp = ps_sm.tile([R, 512], F32, tag="sm", name=f"tp{f}_{ncx}")
                        nc.tensor.matmul(tp[:, :], lhsT=F_sb(f), rhs=uu[0:R, 0, nsl],
                                         start=True, stop=True)
                        nc.vector.tensor_copy(tt[0:R, f, nsl], tp[:, :])
                nc.vector.memset(tt[R:R + 1, 2, :], 1.0)

                # finish qk 0/4: CP add + evac
                for ot in (0, 4):
                    f = 0 if ot < 4 else 1
                    dt_ = ot % 4
                    for ncx in range(NC_CH):
                        nsl = slice(ncx * 512, (ncx + 1) * 512)
                        nc.tensor.matmul(qk04[ot][ncx][:, :], lhsT=cpvw(dt_),
                                         rhs=tt[0:R, f, nsl], start=False, stop=True)
                        nc.vector.tensor_scalar_add(qk[:, ot, nsl], qk04[ot][ncx][:, :],
                                                    cpvb(dt_))
                nc.vector.memset(v_sb[:, :, :, D:D + 1], 1.0)

            # ====== phase 2: attention ======
            # Fine-grained interleave: while the logits/exp stream of pair p
            # runs (exp on ScalarE is the pair-rate limiter), the PE executes
            # attn@v matmuls of pair p-1 (and, for pair 0, the v/qk
            # production) between logits tiles, so neither engine starves.
            # The denominator broadcast is col-tiled into partitions 64:128
            # of the attn@v psum tile (no extra bank).
            with tc.tile_pool(name="ps_log", bufs=2, space="PSUM") as ps_log, \
                 tc.tile_pool(name="ps_av", bufs=3, space="PSUM") as ps_av, \
                 tc.tile_pool(name="ps_bc", bufs=1, space="PSUM") as ps_bc:

                def denom_chain(ap_, hl, pair, ncx):
                    nsl = slice(ncx * 512, (ncx + 1) * 512)
                    den = dnpool.tile([1, 512], BF, tag="den", name=f"den{pair}_{hl}_{ncx}")
                    nc.vector.tensor_copy(den[0:1, :], ap_[D:D + 1, :])
                    bcp = ps_bc.tile([D, 512], F32, tag="bc", name=f"bcp{pair}_{hl}_{ncx}")
                    nc.tensor.matmul(bcp[0:D, :], lhsT=ones_bf[0:1, 0:D],
                                     rhs=den[0:1, :], start=True, stop=True)
                    bc = dnpool.tile([D, 512], F32, tag="bcs", name=f"bc{pair}_{hl}_{ncx}")
                    nc.vector.reciprocal_approx_fast(bc[0:D, :], bcp[0:D, :])
                    nc.vector.tensor_mul(outT[hl * 64:hl * 64 + 64, pair, nsl],
                                         ap_[0:D, :], bc[0:D, :])

                def av_tile(box, key, name):
                    if key not in box:
                        box[key] = ps_av.tile([128, 512], F32, tag="av", name=name)
                    return box[key]

                def make_av_filler(pair, st_pair):
                    # 32 attn@v MMs + 4 denom chains for `pair`, lazy-allocated
                    items = []
                    box = {}
                    for hl in range(2):
                        h = 2 * pair + hl
                        for mt in range(NT):
                            for ncx in range(NC_CH):
                                def mm(hl=hl, mt=mt, ncx=ncx, h=h):
                                    ap_ = av_tile(box, (hl, ncx), f"ap{pair}_{hl}_{ncx}")
                                    nc.tensor.matmul(
                                        ap_[0:D + 1, :], lhsT=v_sb[:, mt, h, :],
                                        rhs=st_pair[hl][:, mt, ncx * 512:(ncx + 1) * 512],
                                        start=(mt == 0), stop=(mt == NT - 1))
                                items.append(mm)
                        for ncx in range(NC_CH):
                            def dn(hl=hl, ncx=ncx):
                                denom_chain(box[(hl, ncx)], hl, pair, ncx)
                            items.append(dn)
                    return items

                def make_p1_filler():
                    # v (m, d) groups + remaining qk tiles under pair-0's stream
                    items = []
                    for mt in range(NT):
                        box = {}
                        for ct in range(CT):
                            def mm(box=box, ct=ct, mt=mt):
                                vp = av_tile(box, "vp", f"vp{mt}")
                                nc.tensor.matmul(
                                    vp[:, :], lhsT=xT[:, ct, mt * 128:(mt + 1) * 128],
                                    rhs=wqkv[:, ct, 2 * DIM:3 * DIM],
                                    start=(ct == 0), stop=False)
                            items.append(mm)
                        def mm2(box=box, mt=mt):
                            nc.tensor.matmul(
                                box["vp"][:, :],
                                lhsT=tt[0:R + 1, 2, mt * 128:(mt + 1) * 128],
                                rhs=cpvw65(), start=False, stop=True)
                        def ev(box=box, mt=mt):
                            nc.vector.tensor_copy(
                                v_sb[:, mt, :, 0:D],
                                box["vp"][:, :].rearrange("p (h d) -> p h d", h=H))
                        items.append(mm2)
                        items.append(ev)
                    for ot in (1, 5, 2, 6, 3, 7):
                        f = 0 if ot < 4 else 1
                        dt_ = ot % 4
                        box = {}
                        for ct in range(CT):
                            for ncx in range(NC_CH):
                                def mm(box=box, ct=ct, ncx=ncx, ot=ot):
                                    qp = av_tile(box, ncx, f"fqp{ot}_{ncx}")
                                    nc.tensor.matmul(
                                        qp[:, :],
                                        lhsT=wqkv[:, ct, ot * 128:(ot + 1) * 128],
                                        rhs=xT[:, ct, ncx * 512:(ncx + 1) * 512],
                                        start=(ct == 0), stop=False)
                                items.append(mm)
                        for ncx in range(NC_CH):
                            def mm2(box=box, ncx=ncx, f=f, dt_=dt_):
                                nc.tensor.matmul(
                                    box[ncx][:, :], lhsT=cpvw(dt_),
                                    rhs=tt[0:R, f, ncx * 512:(ncx + 1) * 512],
                                    start=False, stop=True)
                            def ev(box=box, ncx=ncx, ot=ot, dt_=dt_):
                                nc.vector.tensor_scalar_add(
                                    qk[:, ot, ncx * 512:(ncx + 1) * 512],
                                    box[ncx][:, :], cpvb(dt_))
                            items.append(mm2)
                            items.append(ev)
                    return items

                prev_filler = None
                for pair in range(4):
                    st_h = [stpool.tile([128, NT, N], BF, tag="st", name=f"st_{pair}_{i}")
                            for i in range(2)]
                    slots = [(hl, mt) for hl in range(2) for mt in range(NT)]
                    if pair == 0:
                        prev_filler = make_p1_filler()
                    per_slot = (len(prev_filler) + len(slots) - 1) // len(slots)
                    fi = 0
                    for hl, mt in slots:
                        pb = hl * 64
                        pe = pb + 64
                        msl = slice(mt * 128, (mt + 1) * 128)
                        lp = ps_log.tile([128, N], F32, tag="log")
                        for ncx in range(NC_CH):
                            nsl = slice(ncx * 512, (ncx + 1) * 512)
                            nc.tensor.matmul(lp[:, nsl], lhsT=qk[pb:pe, 4 + pair, msl],
                                             rhs=qk[pb:pe, pair, nsl],
                                             start=True, stop=True,
                                             tile_position=(pb, 0))
                        nc.scalar.activation(st_h[hl][:, mt, :], lp[:, :], AF.Exp,
                                             bias=maskb(mt), scale=SCALE)
                        for _ in range(per_slot):
                            if fi < len(prev_filler):
                                prev_filler[fi]()
                                fi += 1
                    while fi < len(prev_filler):
                        prev_filler[fi]()
                        fi += 1
                    prev_filler = make_av_filler(pair, st_h)

                for item in prev_filler:   # attn@v of pair 3
                    item()

                # ---- u2/t2 + proj ----
                for ncx in range(NC_CH):
                    nsl = slice(ncx * 512, (ncx + 1) * 512)
                    up2 = ps_av.tile([128, 512], F32, tag="av", name=f"up2_{ncx}")
                    for kt in range(CT):
                        nc.tensor.matmul(up2[0:R, :], lhsT=cpuw[:, kt, :],
                                         rhs=outT[:, kt, nsl],
                                         start=(kt == 0), stop=(kt == CT - 1))
                    nc.vector.tensor_scalar_add(uu[0:R, 1, nsl], up2[0:R, :], cpub())
                t2 = work.tile([R, N], BF)
                for ncx in range(NC_CH):
                    nsl = slice(ncx * 512, (ncx + 1) * 512)
                    tp2 = ps_bc.tile([D, 512], F32, tag="bc", name=f"t2p{ncx}")
                    nc.tensor.matmul(tp2[0:R, :], lhsT=F_sb(3), rhs=uu[0:R, 1, nsl],
                                     start=True, stop=True)
                    nc.vector.tensor_copy(t2[0:R, nsl], tp2[0:R, :])

                for ot in range(CT):
                    ppt = ps_log.tile([128, N], F32, tag="log", name=f"ppt{ot}")
                    for kt in range(CT):
                        for ncx in range(NC_CH):
                            nc.tensor.matmul(ppt[:, ncx * 512:(ncx + 1) * 512],
                                             lhsT=wproj[:, kt, ot * 128:(ot + 1) * 128],
                                             rhs=outT[:, kt, ncx * 512:(ncx + 1) * 512],
                                             start=(kt == 0), stop=False)
                    for ncx in range(NC_CH):
                        nsl = slice(ncx * 512, (ncx + 1) * 512)
                        nc.tensor.matmul(ppt[:, nsl], lhsT=cpvw(ot), rhs=t2[0:R, nsl],
                                         start=False, stop=True)
                        po = popool.tile([128, 512], F32, tag="po")
                        nc.vector.tensor_scalar_add(po[:, :], ppt[:, nsl], pbias(ot))
                        eng = nc.sync if (ot + ncx) % 2 == 0 else nc.gpsimd
                        eng.dma_start(out_e[ot * 128:(ot + 1) * 128, nsl], po[:, :])

    nc.compile()
    return nc


def _prep(inputs):
    x = np.asarray(inputs["x"])
    mask = np.asarray(inputs["mask"])
    qkv_w = np.asarray(inputs["qkv_w"], np.float32)
    CP_U_w = np.asarray(inputs["CP_U_w"], np.float32)
    CP_U_b = np.asarray(inputs["CP_U_b"], np.float32)
    CP_V_w = np.asarray(inputs["CP_V_w"], np.float32)
    CP_V_b = np.asarray(inputs["CP_V_b"], np.float32)
    CP_C = np.asarray(inputs["CP_C"], np.float32)
    CP_att = np.asarray(inputs["CP_attention"], np.float32)
    proj_w = np.asarray(inputs["proj_w"], np.float32)
    proj_b = np.asarray(inputs["proj_b"], np.float32)

    fcon = np.zeros((128, 32), np.float32)
    fcon[:, 0:4] = CP_V_b.reshape(CT, 128).T
    fcon[:, 4:8] = (proj_b + CP_V_b).reshape(CT, 128).T
    fcon[0:R, 16] = CP_U_b

    com = {
        "wqkvT": np.ascontiguousarray(qkv_w.T).astype(bf16),
        "cpuwT": np.ascontiguousarray(CP_U_w.T).astype(bf16),
        "cpvwT": np.ascontiguousarray(CP_V_w.T).astype(bf16),
        "cpvw65": np.ascontiguousarray(
            np.concatenate([CP_V_w.T, CP_V_b[None]], 0)).astype(bf16),
        "cpcrt": np.ascontiguousarray(
            np.transpose(CP_C, (2, 0, 1)).reshape(R, R * R)).astype(bf16),
        "cpatt": np.ascontiguousarray(CP_att).astype(bf16),
        "wprojT": np.ascontiguousarray(proj_w.T).astype(bf16),
    }
    in_maps = []
    for b in range(B):
        m = dict(com)
        m["xT"] = np.ascontiguousarray(x[b].T).astype(bf16)
        fc = fcon.copy()
        mb = np.where(mask[b], 0.0, -1e30).astype(np.float32)
        fc[:, 8:16] = mb.reshape(NT, 128).T
        m["fcon"] = fc
        in_maps.append(m)
    return in_maps


LAST_EXEC_NS = None


def kernel(**inputs):
    global LAST_EXEC_NS
    if "nc" not in _CACHE:
        _CACHE["nc"] = _build()
    nc = _CACHE["nc"]
    in_maps = _prep(inputs)
    res = run_bass_kernel_spmd(nc, in_maps, core_ids=list(range(NCORES)))
    LAST_EXEC_NS = res.exec_time_ns
    out = np.stack([np.ascontiguousarray(res.results[i]["out"].T)
                    for i in range(NCORES)])
    return out.astype(np.float32)



# revision 10
# speedup vs baseline: 1.0883x; 1.0883x over previous
"""Trainium2 Bass kernel for nn_CP_Attention_Action (dense transformer block with
CP-factored low-rank corrections).

Data-parallel over batch B=8 -> one batch per NeuronCore, no collectives.

The CP branch is affine in its input, so it is folded into the dense weights on
the host (f64): W_qkv_eff = qkv_w.T + U.T @ F_f @ V.T blocks, with the q/k
biases applied at evacuation and the v bias folded through softmax (weights sum
to 1) into the proj bias. The device then runs a plain attention block:

  qkT (o,n) via stationary W tiles; v in natural (m,d) layout with a ones
  column so the softmax denominator falls out of attn@v; logits via 64x128
  row-group-paired matmuls (both heads of a pair stream concurrently);
  exp on ScalarE in 1536-wide batches (mask is all-ones -> uniform zero bias);
  denominator: reciprocal on DVE + partition_broadcast on GpSimd; proj in
  transposed layout; host transposes (o,n) -> (n,o).

st stream layout per pair: pos = mt*2048 + (ncx*2 + hl)*512 — monotone in the
matmul issue order while alternating PE row groups (hl) between adjacent MMs.
"""

import numpy as np
import ml_dtypes

import concourse.bass as bass
from concourse import bacc
import concourse.mybir as mybir
import concourse.tile as tile
from concourse.bass_utils import run_bass_kernel_spmd

B, N, DIM = 8, 1024, 512
H, D = 8, 64
SCALE = D ** -0.5
NCORES = 8
NC_CH = 2          # n chunks of 512
NT = N // 128      # 8 key tiles
CT = DIM // 128    # 4 c-tiles
OT = 8             # q,k o-tiles
PAIRW = 2 * N      # free elems per key-tile slot in st (2 heads x 1024 q)
STW = NT * PAIRW   # st elems per pair (16384)
LPW = 1536         # activation batch width (3 PSUM banks)
F32 = mybir.dt.float32
BF = mybir.dt.bfloat16
AF = mybir.ActivationFunctionType
bf16 = ml_dtypes.bfloat16

_CACHE = {}


def _stpos(mt, hl, ncx):
    return mt * PAIRW + (ncx * 2 + hl) * 512


def _build(mask_ones: bool):
    nc = bacc.Bacc()

    xT_e = nc.declare_dram_parameter("xT", [DIM, N], BF, isOutput=False)
    wqkvT_e = nc.declare_dram_parameter("wqkvT", [DIM, 3 * DIM], BF, isOutput=False)
    wprojT_e = nc.declare_dram_parameter("wprojT", [DIM, DIM], BF, isOutput=False)
    fcon_e = nc.declare_dram_parameter("fcon", [128, 24], F32, isOutput=False)
    out_e = nc.declare_dram_parameter("out", [DIM, N], F32, isOutput=True)

    # fcon layout (f32 columns): 0:8 qk bias per ot | 8:12 proj bias | 12:20 maskb
    with tile.TileContext(nc) as tc:
        with tc.tile_pool(name="consts", bufs=1) as consts, \
             tc.tile_pool(name="qkpool", bufs=1) as qkpool, \
             tc.tile_pool(name="stpool", bufs=2) as stpool, \
             tc.tile_pool(name="work", bufs=1) as work, \
             tc.tile_pool(name="dnpool", bufs=4) as dnpool, \
             tc.tile_pool(name="bcpool", bufs=2) as bcpool, \
             tc.tile_pool(name="outp", bufs=1) as outp, \
             tc.tile_pool(name="popool", bufs=2) as popool:

            # ---------- inputs (xT + qk weights first) ----------
            warm = consts.tile([128, 512], BF)
            nc.vector.memset(warm[:], 0.0)
            scr = consts.tile([1, 16], BF)
            scr2 = consts.tile([1, 16], BF)
            nc.vector.memset(scr[:], 0.0)
            xT = consts.tile([128, CT, N], BF)
            nc.sync.dma_start(xT[:], xT_e[:, :].rearrange("(ct p) n -> p ct n", p=128))
            fcon = consts.tile([128, 24], F32)
            nc.sync.dma_start(fcon[:], fcon_e[:, :])
            wqkv = consts.tile([128, CT, 3 * DIM], BF)
            nc.gpsimd.dma_start(
                wqkv[:, :, 0:2 * DIM],
                wqkvT_e[0:DIM, 0:2 * DIM].rearrange("(ct p) o -> p ct o", p=128))
            nc.gpsimd.dma_start(
                wqkv[:, :, 2 * DIM:3 * DIM],
                wqkvT_e[0:DIM, 2 * DIM:3 * DIM].rearrange("(ct p) o -> p ct o", p=128))
            wproj = consts.tile([128, CT, DIM], BF)
            nc.gpsimd.dma_start(wproj[:], wprojT_e[:, :].rearrange("(ct p) o -> p ct o", p=128))

            def qkbias(ot):
                return fcon[:, ot:ot + 1]

            def pbias(ot):
                return fcon[:, 8 + ot:9 + ot]

            def maskb(mt):
                return fcon[:, 12 + mt:13 + mt]

            outT = outp.tile([128, CT, N], BF)
            v_sb = work.tile([128, NT, H, D + 1], BF)
            qk = qkpool.tile([128, OT, N], BF)
            nc.vector.memset(v_sb[:, :, :, D:D + 1], 1.0)
            # early exp-table load on a scratch tile (overlaps the DMA wait)
            nc.scalar.activation(scr2[:], scr[:], AF.Exp, bias=0.0, scale=1.0)

            # ---------- warm-up matmuls (HAM un-throttle during DMA wait) ----------
            wscr = consts.tile([1, 16], F32)
            with tc.tile_pool(name="wps", bufs=1, space="PSUM") as wps:
                wp = wps.tile([128, 512], F32)
                for i in range(9):
                    nc.tensor.matmul(wp[:, :], lhsT=warm[:, 0:128], rhs=warm[:, :],
                                     start=(i == 0), stop=(i == 8))
                # reader so later pools' bank reuse orders after the warm-up
                nc.vector.tensor_copy(wscr[0:1, :], wp[0:1, 0:16])

            with tc.tile_pool(name="lpp", bufs=2, space="PSUM") as lpp, \
                 tc.tile_pool(name="avp", bufs=2, space="PSUM") as avp:

                def qk_items(ot):
                    items = []
                    for ncx in range(NC_CH):
                        box = {}

                        def mk(ct, box=box, ncx=ncx, ot=ot):
                            def mm():
                                if "t" not in box:
                                    box["t"] = avp.tile([128, 512], F32, tag="av",
                                                        name=f"qp{ot}_{ncx}")
                                nc.tensor.matmul(
                                    box["t"][:, :],
                                    lhsT=wqkv[:, ct, ot * 128:(ot + 1) * 128],
                                    rhs=xT[:, ct, ncx * 512:(ncx + 1) * 512],
                                    start=(ct == 0), stop=(ct == CT - 1))
                            return mm

                        def ev(box=box, ncx=ncx, ot=ot):
                            nc.vector.tensor_scalar_add(
                                qk[:, ot, ncx * 512:(ncx + 1) * 512],
                                box["t"][:, :], qkbias(ot))
                        for ct in range(CT):
                            items.append(mk(ct))
                        items.append(ev)
                    return items

                # qk pair-0 weights first so pair 0 can start immediately
                for it in qk_items(0) + qk_items(4):
                    it()

                def v_items():
                    items = []
                    for mt in range(NT):
                        box = {}

                        def mk(ct, box=box, mt=mt):
                            def mm():
                                if "t" not in box:
                                    box["t"] = avp.tile([128, 512], F32, tag="av",
                                                        name=f"vp{mt}")
                                nc.tensor.matmul(
                                    box["t"][:, :],
                                    lhsT=xT[:, ct, mt * 128:(mt + 1) * 128],
                                    rhs=wqkv[:, ct, 2 * DIM:3 * DIM],
                                    start=(ct == 0), stop=(ct == CT - 1))
                            return mm

                        def ev(box=box, mt=mt):
                            nc.vector.tensor_copy(
                                v_sb[:, mt, :, 0:D],
                                box["t"][:, :].rearrange("p (h d) -> p h d", h=H))
                        for ct in range(CT):
                            items.append(mk(ct))
                        items.append(ev)
                    return items

                # ---------- attention ----------
                st_t = [None, None]

                def av_items(pair):
                    # attn@v of `pair` + denominator chains, reading st_t[pair%2]
                    items = []
                    stp = st_t[pair % 2]
                    for ncx in range(NC_CH):
                        for hl in range(2):
                            h = 2 * pair + hl
                            box = {}

                            def mk(mt, box=box, hl=hl, ncx=ncx, h=h):
                                def mm():
                                    if "t" not in box:
                                        box["t"] = avp.tile([128, 512], F32, tag="av",
                                                            name=f"ap{h}_{ncx}")
                                    p0 = _stpos(mt, hl, ncx)
                                    nc.tensor.matmul(
                                        box["t"][0:D + 1, :],
                                        lhsT=v_sb[:, mt, h, :],
                                        rhs=stp[:, p0:p0 + 512],
                                        start=(mt == 0), stop=(mt == NT - 1))
                                return mm

                            def chain(box=box, hl=hl, ncx=ncx, pair=pair, h=h):
                                # custom DVE ops drop the input base partition:
                                # copy the den row to partition 0 before recip
                                dnr = dnpool.tile([1, 512], F32, tag="dnr",
                                                  name=f"dnr{h}_{ncx}")
                                nc.vector.tensor_copy(dnr[0:1, :],
                                                      box["t"][D:D + 1, :])
                                dn = dnpool.tile([1, 512], F32, tag="dn",
                                                 name=f"dn{h}_{ncx}")
                                nc.vector.reciprocal_approx_fast(
                                    dn[0:1, :], dnr[0:1, :])
                                bc = bcpool.tile([D, 512], F32, tag="bc",
                                                 name=f"bc{h}_{ncx}")
                                nc.gpsimd.partition_broadcast(bc[:, :], dn[0:1, :])
                                nc.vector.tensor_mul(
                                    outT[hl * 64:hl * 64 + 64, pair,
                                         ncx * 512:(ncx + 1) * 512],
                                    box["t"][0:D, :], bc[:, :])
                            for mt in range(NT):
                                items.append(mk(mt))
                            items.append(chain)
                    return items

                filler = []
                fi = 0

                def consume(k):
                    nonlocal fi
                    e = min(fi + k, len(filler))
                    while fi < e:
                        filler[fi]()
                        fi += 1

                for pair in range(4):
                    stp = stpool.tile([128, STW], BF, tag="st", name=f"st{pair % 2}")
                    st_t[pair % 2] = stp
                    if pair == 0:
                        filler = v_items()
                        for ot in (1, 5, 2, 6, 3, 7):
                            filler += qk_items(ot)
                        fi = 0
                    per = max(1, (len(filler) - fi + 31) // 32)

                    lpt = {}
                    acted = 0
                    for mt in range(NT):
                        for hl, ncx in ((0, 0), (1, 0), (0, 1), (1, 1)):
                            pos = _stpos(mt, hl, ncx)
                            ti = pos // LPW
                            off = pos - ti * LPW
                            if ti not in lpt:
                                lpt[ti] = lpp.tile([128, LPW], F32, tag="lp",
                                                   name=f"lp{pair}_{ti % 2}")
                            pb = hl * 64
                            nc.tensor.matmul(
                                lpt[ti][:, off:off + 512],
                                lhsT=qk[pb:pb + 64, 4 + pair, mt * 128:(mt + 1) * 128],
                                rhs=qk[pb:pb + 64, pair, ncx * 512:(ncx + 1) * 512],
                                start=True, stop=True, tile_position=(pb, 0))
                            if not mask_ones:
                                nc.scalar.activation(
                                    stp[:, pos:pos + 512], lpt[ti][:, off:off + 512],
                                    AF.Exp, bias=maskb(mt), scale=SCALE)
                                if off + 512 == LPW or pos + 512 == STW:
                                    del lpt[ti]
                                    acted = ti + 1
                            else:
                                # fire activation for each fully-written 1536 chunk
                                while acted * LPW + LPW <= pos + 512 or \
                                        pos + 512 == STW:
                                    w = min(LPW, STW - acted * LPW)
                                    nc.scalar.activation(
                                        stp[:, acted * LPW:acted * LPW + w],
                                        lpt[acted][:, 0:w], AF.Exp,
                                        bias=0.0, scale=SCALE)
                                    del lpt[acted]
                                    acted += 1
                                    if acted * LPW >= STW:
                                        break
                            consume(per)

                    consume(len(filler))
                    filler = av_items(pair)
                    fi = 0

                consume(len(filler))   # attn@v of pair 3

                # ---------- proj ----------
                import os as _os
                if _os.environ.get("BASSDEBUG") == "qk":
                    for ncx in range(NC_CH):
                        nsl = slice(ncx * 512, (ncx + 1) * 512)
                        for kt in range(CT):
                            po = popool.tile([128, 512], F32, tag="po")
                            nc.vector.tensor_copy(po[:, :], qk[:, kt, nsl])
                            nc.sync.dma_start(out_e[kt * 128:(kt + 1) * 128, nsl],
                                              po[:, :])
                elif _os.environ.get("BASSDEBUG") == "st":
                    for mt in range(4):
                        po = popool.tile([128, 512], F32, tag="po")
                        nc.vector.tensor_copy(po[:, :], st_t[1][:, mt * PAIRW:mt * PAIRW + 512])
                        nc.sync.dma_start(out_e[mt * 128:(mt + 1) * 128, 0:512], po[:, :])
                        po2 = popool.tile([128, 512], F32, tag="po")
                        nc.vector.tensor_copy(po2[:, :], st_t[1][:, mt * PAIRW + 1024:mt * PAIRW + 1536])
                        nc.sync.dma_start(out_e[mt * 128:(mt + 1) * 128, 512:1024], po2[:, :])
                elif _os.environ.get("BASSDEBUG") == "outT":
                    for ncx in range(NC_CH):
                        nsl = slice(ncx * 512, (ncx + 1) * 512)
                        for kt in range(CT):
                            po = popool.tile([128, 512], F32, tag="po")
                            nc.vector.tensor_copy(po[:, :], outT[:, kt, nsl])
                            nc.sync.dma_start(out_e[kt * 128:(kt + 1) * 128, nsl],
                                              po[:, :])
                else:
                    for ncx in range(NC_CH):
                        nsl = slice(ncx * 512, (ncx + 1) * 512)
                        for ot in range(CT):
                            pp = lpp.tile([128, LPW], F32, tag="lp", name=f"pp{ot % 2}")
                            for kt in range(CT):
                                nc.tensor.matmul(pp[:, 0:512],
                                                 lhsT=wproj[:, kt, ot * 128:(ot + 1) * 128],
                                                 rhs=outT[:, kt, nsl],
                                                 start=(kt == 0), stop=(kt == CT - 1))
                            po = popool.tile([128, 512], F32, tag="po")
                            nc.vector.tensor_scalar_add(po[:, :], pp[:, 0:512], pbias(ot))
                            deng = nc.sync if (ot + ncx) % 2 == 0 else nc.gpsimd
                            deng.dma_start(out_e[ot * 128:(ot + 1) * 128, nsl], po[:, :])

    nc.compile()
    return nc


def _prep(inputs):
    x = np.asarray(inputs["x"])
    mask = np.asarray(inputs["mask"])
    qkv_w = np.asarray(inputs["qkv_w"], np.float64)
    CP_U_w = np.asarray(inputs["CP_U_w"], np.float64)
    CP_U_b = np.asarray(inputs["CP_U_b"], np.float64)
    CP_V_w = np.asarray(inputs["CP_V_w"], np.float64)
    CP_V_b = np.asarray(inputs["CP_V_b"], np.float64)
    CP_C = np.asarray(inputs["CP_C"], np.float64)
    CP_att = np.asarray(inputs["CP_attention"], np.float64)
    proj_w = np.asarray(inputs["proj_w"], np.float64)
    proj_b = np.asarray(inputs["proj_b"], np.float64)

    # fold the CP branch (affine in its input) into the dense weights
    F = np.einsum('ijr,rf->fij', CP_C, CP_att)          # (4, R, R)
    UT = CP_U_w.T                                        # (DIM, R)
    VT = CP_V_w.T                                        # (R, DIM)
    A = np.stack([UT @ F[f] @ VT for f in range(4)])     # (4, DIM, DIM)
    c = np.stack([CP_U_b @ F[f] @ VT + CP_V_b for f in range(4)])  # (4, DIM)

    Wqkv = qkv_w.T + np.concatenate([A[0], A[1], A[2]], axis=1)   # (DIM, 3*DIM)
    Wp = proj_w.T + A[3]                                          # (DIM, DIM)
    b_qk = np.concatenate([c[0], c[1]])                           # (2*DIM,)
    b_out = proj_b + c[3] + c[2] @ Wp                             # (DIM,)

    fcon = np.zeros((128, 24), np.float32)
    fcon[:, 0:8] = b_qk.reshape(OT, 128).T
    fcon[:, 8:12] = b_out.reshape(CT, 128).T

    mask_ones = bool(mask.all())
    com = {
        "wqkvT": np.ascontiguousarray(Wqkv).astype(bf16),
        "wprojT": np.ascontiguousarray(Wp).astype(bf16),
    }
    in_maps = []
    for b in range(B):
        m = dict(com)
        m["xT"] = np.ascontiguousarray(x[b].T).astype(bf16)
        fc = fcon.copy()
        if not mask_ones:
            mb = np.where(mask[b], 0.0, -1e30).astype(np.float32)
            fc[:, 12:20] = mb.reshape(NT, 128).T
        m["fcon"] = fc
        in_maps.append(m)
    return in_maps, mask_ones


LAST_EXEC_NS = None


def kernel(**inputs):
    global LAST_EXEC_NS
    in_maps, mask_ones = _prep(inputs)
    key = ("nc", mask_ones)
    if key not in _CACHE:
        _CACHE[key] = _build(mask_ones)
    nc = _CACHE[key]
    res = run_bass_kernel_spmd(nc, in_maps, core_ids=list(range(NCORES)))
    LAST_EXEC_NS = res.exec_time_ns
    out = np.stack([np.ascontiguousarray(res.results[i]["out"].T)
                    for i in range(NCORES)])
    return out.astype(np.float32)


# revision 12
# speedup vs baseline: 1.2223x; 1.1231x over previous
"""Trainium2 Bass kernel for nn_CP_Attention_Action (dense transformer block with
CP-factored low-rank corrections).

Data-parallel over batch B=8 -> one batch per NeuronCore, no collectives.

The CP branch is affine in its input, so it is folded into the dense weights on
the host (f64): W_qkv_eff = qkv_w.T + U.T @ F_f @ V.T blocks, with the q/k
biases applied at evacuation and the v bias folded through softmax (weights sum
to 1) into the proj bias. The device then runs a plain attention block:

  qkT (o,n) via stationary W tiles; v in natural (m,d) layout with a ones
  column so the softmax denominator falls out of attn@v; logits via 64x128
  row-group-paired matmuls (hl0/hl1 issued adjacently -> both stream on PE
  concurrently); exp on ScalarE in 1536-wide batches (all-ones mask -> uniform
  zero bias); denominator: copy+reciprocal on DVE + partition_broadcast on
  GpSimd; attn@v of pair p runs as PE filler during pair p+1; pair 3 uses an
  ncx-major st layout so its ncx0 attn@v half completes mid-pair and only the
  ncx1 half + proj remain in the tail. Host pre-arranges all inputs into
  partition-major layout for fat contiguous DMA descriptors on 4 queues.
"""

import os
import numpy as np
import ml_dtypes

from concourse import bacc
import concourse.mybir as mybir
import concourse.tile as tile
from concourse.bass_utils import run_bass_kernel_spmd

B, N, DIM = 8, 1024, 512
H, D = 8, 64
SCALE = D ** -0.5
NCORES = 8
NC_CH = 2          # n chunks of 512
NT = N // 128      # 8 key tiles
CT = DIM // 128    # 4 c-tiles
OT = 8             # q,k o-tiles
PAIRW = 2 * N      # free elems per key-tile slot in st (2 heads x 1024 q)
STW = NT * PAIRW   # st elems per pair (16384)
LPW = 1536         # activation batch width (3 PSUM banks)
F32 = mybir.dt.float32
BF = mybir.dt.bfloat16
AF = mybir.ActivationFunctionType
bf16 = ml_dtypes.bfloat16

_CACHE = {}


def _stpos(pair, mt, hl, ncx):
    if pair < 3:
        return mt * PAIRW + (ncx * 2 + hl) * 512
    return ncx * (NT * 1024) + mt * 1024 + hl * 512


def _slots(pair):
    # (mt, hl, ncx) in ascending stream-position order, hl adjacent
    out = []
    if pair < 3:
        for mt in range(NT):
            for ncx in range(NC_CH):
                for hl in range(2):
                    out.append((mt, hl, ncx))
    else:
        for ncx in range(NC_CH):
            for mt in range(NT):
                for hl in range(2):
                    out.append((mt, hl, ncx))
    return out


def _build(mask_ones: bool):
    nc = bacc.Bacc()

    xT_e = nc.declare_dram_parameter("xT", [128, CT, N], BF, isOutput=False)
    wqkA_e = nc.declare_dram_parameter("wqkA", [128, CT, 2 * DIM], BF, isOutput=False)
    wvB_e = nc.declare_dram_parameter("wvB", [128, CT, DIM], BF, isOutput=False)
    wproj_e = nc.declare_dram_parameter("wproj", [128, CT, DIM], BF, isOutput=False)
    fcon_e = nc.declare_dram_parameter("fcon", [128, 24], F32, isOutput=False)
    out_e = nc.declare_dram_parameter("out", [DIM, N], F32, isOutput=True)

    # fcon layout (f32 columns): 0:8 qk bias per ot | 8:12 proj bias | 12:20 maskb
    with tile.TileContext(nc) as tc:
        with tc.tile_pool(name="consts", bufs=1) as consts, \
             tc.tile_pool(name="qkpool", bufs=1) as qkpool, \
             tc.tile_pool(name="stpool", bufs=2) as stpool, \
             tc.tile_pool(name="work", bufs=1) as work, \
             tc.tile_pool(name="dnpool", bufs=4) as dnpool, \
             tc.tile_pool(name="bcpool", bufs=2) as bcpool, \
             tc.tile_pool(name="outp", bufs=1) as outp, \
             tc.tile_pool(name="popool", bufs=2) as popool:

            # ---------- inputs: 4 DMA queues, partition-major layouts ----------
            warm = consts.tile([128, 512], BF)
            nc.vector.memset(warm[:], 0.0)
            scr = consts.tile([1, 16], BF)
            scr2 = consts.tile([1, 16], BF)
            nc.vector.memset(scr[:], 0.0)
            fcon = consts.tile([128, 24], F32)
            nc.sync.dma_start(fcon[:], fcon_e[:, :])
            xT = consts.tile([128, CT, N], BF)
            nc.sync.dma_start(xT[:], xT_e[:, :, :])
            wqkv = consts.tile([128, CT, 3 * DIM], BF)
            nc.gpsimd.dma_start(wqkv[:, :, 0:2 * DIM], wqkA_e[:, :, :])
            nc.scalar.dma_start(wqkv[:, :, 2 * DIM:3 * DIM], wvB_e[:, :, :])
            wproj = consts.tile([128, CT, DIM], BF)
            nc.scalar.dma_start(wproj[:], wproj_e[:, :, :])

            def qkbias(ot):
                return fcon[:, ot:ot + 1]

            def pbias(ot):
                return fcon[:, 8 + ot:9 + ot]

            def maskb(mt):
                return fcon[:, 12 + mt:13 + mt]

            outT = outp.tile([128, CT, N], BF)
            v_sb = work.tile([128, NT, H, D + 1], BF)
            qk = qkpool.tile([128, OT, N], BF)
            nc.vector.memset(v_sb[:, :, :, D:D + 1], 1.0)
            # early exp-table load on a scratch tile (overlaps the DMA wait)
            nc.scalar.activation(scr2[:], scr[:], AF.Exp, bias=0.0, scale=1.0)

            # ---------- warm-up matmuls (HAM un-throttle during DMA wait) ----------
            wscr = consts.tile([1, 16], F32)
            with tc.tile_pool(name="wps", bufs=1, space="PSUM") as wps:
                wp = wps.tile([128, 512], F32)
                for i in range(8):
                    nc.tensor.matmul(wp[:, :], lhsT=warm[:, 0:128], rhs=warm[:, :],
                                     start=(i == 0), stop=(i == 7))
                # reader so later pools' bank reuse orders after the warm-up
                nc.vector.tensor_copy(wscr[0:1, :], wp[0:1, 0:16])

            with tc.tile_pool(name="lpp", bufs=2, space="PSUM") as lpp, \
                 tc.tile_pool(name="avp", bufs=2, space="PSUM") as avp:

                def qk_items(ot):
                    items = []
                    for ncx in range(NC_CH):
                        box = {}

                        def mk(ct, box=box, ncx=ncx, ot=ot):
                            def mm():
                                if "t" not in box:
                                    box["t"] = avp.tile([128, 512], F32, tag="av",
                                                        name=f"qp{ot}_{ncx}")
                                nc.tensor.matmul(
                                    box["t"][:, :],
                                    lhsT=wqkv[:, ct, ot * 128:(ot + 1) * 128],
                                    rhs=xT[:, ct, ncx * 512:(ncx + 1) * 512],
                                    start=(ct == 0), stop=(ct == CT - 1))
                            return mm

                        def ev(box=box, ncx=ncx, ot=ot):
                            nc.vector.tensor_scalar_add(
                                qk[:, ot, ncx * 512:(ncx + 1) * 512],
                                box["t"][:, :], qkbias(ot))
                        for ct in range(CT):
                            items.append(mk(ct))
                        items.append(ev)
                    return items

                # qk pair-0 weights first so pair 0 can start immediately
                for it in qk_items(0) + qk_items(4):
                    it()

                def v_items():
                    items = []
                    for mt in range(NT):
                        box = {}

                        def mk(ct, box=box, mt=mt):
                            def mm():
                                if "t" not in box:
                                    box["t"] = avp.tile([128, 512], F32, tag="av",
                                                        name=f"vp{mt}")
                                nc.tensor.matmul(
                                    box["t"][:, :],
                                    lhsT=xT[:, ct, mt * 128:(mt + 1) * 128],
                                    rhs=wqkv[:, ct, 2 * DIM:3 * DIM],
                                    start=(ct == 0), stop=(ct == CT - 1))
                            return mm

                        def ev(box=box, mt=mt):
                            nc.vector.tensor_copy(
                                v_sb[:, mt, :, 0:D],
                                box["t"][:, :].rearrange("p (h d) -> p h d", h=H))
                        for ct in range(CT):
                            items.append(mk(ct))
                        items.append(ev)
                    return items

                # ---------- attention ----------
                st_t = [None, None]

                def av_group(pair, hl, ncx):
                    # attn@v accumulation + denominator chain for one head/ncx
                    items = []
                    stp = st_t[pair % 2]
                    h = 2 * pair + hl
                    box = {}

                    def mk(mt, box=box, hl=hl, ncx=ncx, h=h, pair=pair):
                        def mm():
                            if "t" not in box:
                                box["t"] = avp.tile([128, 512], F32, tag="av",
                                                    name=f"ap{h}_{ncx}")
                            p0 = _stpos(pair, mt, hl, ncx)
                            nc.tensor.matmul(
                                box["t"][0:D + 1, :],
                                lhsT=v_sb[:, mt, h, :],
                                rhs=stp[:, p0:p0 + 512],
                                start=(mt == 0), stop=(mt == NT - 1))
                        return mm

                    def chain(box=box, hl=hl, ncx=ncx, pair=pair, h=h):
                        # custom DVE ops drop the input base partition: copy the
                        # den row to partition 0 before the reciprocal
                        dnr = dnpool.tile([1, 512], F32, tag="dnr",
                                          name=f"dnr{h}_{ncx}")
                        nc.vector.tensor_copy(dnr[0:1, :], box["t"][D:D + 1, :])
                        dn = dnpool.tile([1, 512], F32, tag="dn",
                                         name=f"dn{h}_{ncx}")
                        nc.vector.reciprocal_approx_fast(dn[0:1, :], dnr[0:1, :])
                        bc = bcpool.tile([D, 512], F32, tag="bc",
                                         name=f"bc{h}_{ncx}")
                        nc.gpsimd.partition_broadcast(bc[:, :], dn[0:1, :])
                        nc.vector.tensor_mul(
                            outT[hl * 64:hl * 64 + 64, pair,
                                 ncx * 512:(ncx + 1) * 512],
                            box["t"][0:D, :], bc[:, :])
                    for mt in range(NT):
                        items.append(mk(mt))
                    items.append(chain)
                    return items

                def proj_items(ncx):
                    items = []
                    nsl = slice(ncx * 512, (ncx + 1) * 512)
                    for ot in range(CT):
                        box = {}

                        def mk(kt, box=box, ot=ot, ncx=ncx):
                            def mm():
                                if "t" not in box:
                                    box["t"] = lpp.tile([128, LPW], F32, tag="lp",
                                                        name=f"pp{ot % 2}")
                                nc.tensor.matmul(
                                    box["t"][:, 0:512],
                                    lhsT=wproj[:, kt, ot * 128:(ot + 1) * 128],
                                    rhs=outT[:, kt, ncx * 512:(ncx + 1) * 512],
                                    start=(kt == 0), stop=(kt == CT - 1))
                            return mm

                        def ev(box=box, ot=ot, ncx=ncx, nsl=nsl):
                            po = popool.tile([128, 512], F32, tag="po")
                            nc.vector.tensor_scalar_add(po[:, :], box["t"][:, 0:512],
                                                        pbias(ot))
                            deng = nc.sync if (ot + ncx) % 2 == 0 else nc.gpsimd
                            deng.dma_start(out_e[ot * 128:(ot + 1) * 128, nsl],
                                           po[:, :])
                        for kt in range(CT):
                            items.append(mk(kt))
                        items.append(ev)
                    return items

                filler = []
                fi = 0

                def consume(k):
                    nonlocal fi
                    e = min(fi + k, len(filler))
                    while fi < e:
                        filler[fi]()
                        fi += 1

                for pair in range(4):
                    stp = stpool.tile([128, STW], BF, tag="st", name=f"st{pair % 2}")
                    st_t[pair % 2] = stp
                    if pair == 0:
                        filler = v_items()
                        for ot in (1, 5, 2, 6, 3, 7):
                            filler += qk_items(ot)
                        fi = 0
                    elif pair == 3:
                        # av(2) plus the ncx0 half of av(3) (ncx-major st layout
                        # means its chunks complete in the first half of pair 3)
                        rem = filler[fi:]
                        filler = rem + av_group(3, 0, 0) + av_group(3, 1, 0)
                        fi = 0
                    npts = len(_slots(pair)) // 2
                    per = max(1, (len(filler) - fi + npts - 1) // npts)

                    lpt = {}
                    acted = 0
                    for si, (mt, hl, ncx) in enumerate(_slots(pair)):
                        pos = _stpos(pair, mt, hl, ncx)
                        ti = pos // LPW
                        off = pos - ti * LPW
                        if ti not in lpt:
                            lpt[ti] = lpp.tile([128, LPW], F32, tag="lp",
                                               name=f"lp{pair}_{ti % 2}")
                        pb = hl * 64
                        nc.tensor.matmul(
                            lpt[ti][:, off:off + 512],
                            lhsT=qk[pb:pb + 64, 4 + pair, mt * 128:(mt + 1) * 128],
                            rhs=qk[pb:pb + 64, pair, ncx * 512:(ncx + 1) * 512],
                            start=True, stop=True, tile_position=(pb, 0))
                        if not mask_ones:
                            nc.scalar.activation(
                                stp[:, pos:pos + 512], lpt[ti][:, off:off + 512],
                                AF.Exp, bias=maskb(mt), scale=SCALE)
                            if off + 512 == LPW or pos + 512 == STW:
                                del lpt[ti]
                        else:
                            while acted * LPW + LPW <= pos + 512 or \
                                    pos + 512 == STW:
                                w = min(LPW, STW - acted * LPW)
                                nc.scalar.activation(
                                    stp[:, acted * LPW:acted * LPW + w],
                                    lpt[acted][:, 0:w], AF.Exp,
                                    bias=0.0, scale=SCALE)
                                del lpt[acted]
                                acted += 1
                                if acted * LPW >= STW:
                                    break
                        if si % 2 == 1:
                            consume(per)   # only between hl pairs (PE row pairing)

                    consume(len(filler))
                    if pair < 3:
                        filler = []
                        for ncx in range(NC_CH):
                            for hl in range(2):
                                filler += av_group(pair, hl, ncx)
                        fi = 0

                # ---------- tail: av(3) ncx1 + proj ----------
                filler = av_group(3, 0, 1) + av_group(3, 1, 1)
                fi = 0
                tail = proj_items(0)
                ti2 = 0
                for it in filler:
                    it()
                    # interleave proj(ncx0) (independent of av(3) ncx1)
                    if ti2 < len(tail):
                        tail[ti2]()
                        ti2 += 1
                while ti2 < len(tail):
                    tail[ti2]()
                    ti2 += 1
                for it in proj_items(1):
                    it()

    nc.compile()
    return nc


def _prep(inputs):
    x = np.asarray(inputs["x"])
    mask = np.asarray(inputs["mask"])
    qkv_w = np.asarray(inputs["qkv_w"], np.float64)
    CP_U_w = np.asarray(inputs["CP_U_w"], np.float64)
    CP_U_b = np.asarray(inputs["CP_U_b"], np.float64)
    CP_V_w = np.asarray(inputs["CP_V_w"], np.float64)
    CP_V_b = np.asarray(inputs["CP_V_b"], np.float64)
    CP_C = np.asarray(inputs["CP_C"], np.float64)
    CP_att = np.asarray(inputs["CP_attention"], np.float64)
    proj_w = np.asarray(inputs["proj_w"], np.float64)
    proj_b = np.asarray(inputs["proj_b"], np.float64)

    # fold the CP branch (affine in its input) into the dense weights
    F = np.einsum('ijr,rf->fij', CP_C, CP_att)          # (4, R, R)
    UT = CP_U_w.T                                        # (DIM, R)
    VT = CP_V_w.T                                        # (R, DIM)
    A = np.stack([UT @ F[f] @ VT for f in range(4)])     # (4, DIM, DIM)
    c = np.stack([CP_U_b @ F[f] @ VT + CP_V_b for f in range(4)])  # (4, DIM)

    Wqkv = qkv_w.T + np.concatenate([A[0], A[1], A[2]], axis=1)   # (DIM, 3*DIM)
    Wp = proj_w.T + A[3]                                          # (DIM, DIM)
    b_qk = np.concatenate([c[0], c[1]])                           # (2*DIM,)
    b_out = proj_b + c[3] + c[2] @ Wp                             # (DIM,)

    fcon = np.zeros((128, 24), np.float32)
    fcon[:, 0:8] = b_qk.reshape(OT, 128).T
    fcon[:, 8:12] = b_out.reshape(CT, 128).T

    def pmajor(w):
        # (DIM, W) -> (128, CT, W): partition p holds rows {p, 128+p, ...}
        return np.ascontiguousarray(
            w.reshape(CT, 128, w.shape[1]).transpose(1, 0, 2))

    mask_ones = bool(mask.all())
    com = {
        "wqkA": pmajor(Wqkv[:, 0:2 * DIM]).astype(bf16),
        "wvB": pmajor(Wqkv[:, 2 * DIM:3 * DIM]).astype(bf16),
        "wproj": pmajor(Wp).astype(bf16),
    }
    in_maps = []
    for b in range(B):
        m = dict(com)
        m["xT"] = pmajor(x[b].T.astype(np.float64)).astype(bf16)
        fc = fcon.copy()
        if not mask_ones:
            mb = np.where(mask[b], 0.0, -1e30).astype(np.float32)
            fc[:, 12:20] = mb.reshape(NT, 128).T
        m["fcon"] = fc
        in_maps.append(m)
    return in_maps, mask_ones


LAST_EXEC_NS = None


def kernel(**inputs):
    global LAST_EXEC_NS
    in_maps, mask_ones = _prep(inputs)
    key = ("nc", mask_ones)
    if key not in _CACHE:
        _CACHE[key] = _build(mask_ones)
    nc = _CACHE[key]
    res = run_bass_kernel_spmd(nc, in_maps, core_ids=list(range(NCORES)))
    LAST_EXEC_NS = res.exec_time_ns
    out = np.stack([np.ascontiguousarray(res.results[i]["out"].T)
                    for i in range(NCORES)])
    return out.astype(np.float32)


# revision 13
# speedup vs baseline: 1.2495x; 1.0223x over previous
"""Trainium2 Bass kernel for nn_CP_Attention_Action (dense transformer block with
CP-factored low-rank corrections).

Data-parallel over batch B=8 -> one batch per NeuronCore, no collectives.

The CP branch is affine in its input, so it is folded into the dense weights on
the host (f64): W_qkv_eff = qkv_w.T + U.T @ F_f @ V.T blocks, with the q/k
biases applied at evacuation and the v bias folded through softmax (weights sum
to 1) into the proj bias. The device then runs a plain attention block:

  qkT (o,n) via stationary W tiles; v in natural (m,d) layout with a ones
  column so the softmax denominator falls out of attn@v; logits via 64x128
  row-group-paired matmuls (hl0/hl1 issued adjacently -> both stream on PE
  concurrently); exp on ScalarE in 1536-wide batches (all-ones mask -> uniform
  zero bias); denominator: copy+reciprocal on DVE + partition_broadcast on
  GpSimd; attn@v of pair p runs as PE filler during pair p+1; pair 3 uses an
  ncx-major st layout so its ncx0 attn@v half completes mid-pair and only the
  ncx1 half + proj remain in the tail. Host pre-arranges all inputs into
  partition-major layout for fat contiguous DMA descriptors on 4 queues.
"""

import os
import numpy as np
import ml_dtypes

from concourse import bacc
import concourse.mybir as mybir
import concourse.tile as tile
from concourse.bass_utils import run_bass_kernel_spmd

B, N, DIM = 8, 1024, 512
H, D = 8, 64
SCALE = D ** -0.5
NCORES = 8
NC_CH = 2          # n chunks of 512
NT = N // 128      # 8 key tiles
CT = DIM // 128    # 4 c-tiles
OT = 8             # q,k o-tiles
PAIRW = 2 * N      # free elems per key-tile slot in st (2 heads x 1024 q)
STW = NT * PAIRW   # st elems per pair (16384)
LPW = 1536         # activation batch width (3 PSUM banks)
F32 = mybir.dt.float32
BF = mybir.dt.bfloat16
AF = mybir.ActivationFunctionType
bf16 = ml_dtypes.bfloat16

_CACHE = {}


def _stpos(pair, mt, hl, ncx):
    if pair < 3:
        return mt * PAIRW + (ncx * 2 + hl) * 512
    return ncx * (NT * 1024) + mt * 1024 + hl * 512


def _slots(pair):
    # (mt, hl, ncx) in ascending stream-position order, hl adjacent
    out = []
    if pair < 3:
        for mt in range(NT):
            for ncx in range(NC_CH):
                for hl in range(2):
                    out.append((mt, hl, ncx))
    else:
        for ncx in range(NC_CH):
            for mt in range(NT):
                for hl in range(2):
                    out.append((mt, hl, ncx))
    return out


def _build(mask_ones: bool):
    nc = bacc.Bacc()

    xT_e = nc.declare_dram_parameter("xT", [128, CT, N], BF, isOutput=False)
    wqkA_e = nc.declare_dram_parameter("wqkA", [128, CT, 2 * DIM], BF, isOutput=False)
    wvB_e = nc.declare_dram_parameter("wvB", [128, CT, DIM], BF, isOutput=False)
    wproj_e = nc.declare_dram_parameter("wproj", [128, CT, DIM], BF, isOutput=False)
    fcon_e = nc.declare_dram_parameter("fcon", [128, 24], F32, isOutput=False)
    out_e = nc.declare_dram_parameter("out", [DIM, N], F32, isOutput=True)

    # fcon layout (f32 columns): 0:8 qk bias per ot | 8:12 proj bias | 12:20 maskb
    with tile.TileContext(nc) as tc:
        with tc.tile_pool(name="consts", bufs=1) as consts, \
             tc.tile_pool(name="qkpool", bufs=1) as qkpool, \
             tc.tile_pool(name="stpool", bufs=2) as stpool, \
             tc.tile_pool(name="work", bufs=1) as work, \
             tc.tile_pool(name="dnpool", bufs=4) as dnpool, \
             tc.tile_pool(name="bcpool", bufs=2) as bcpool, \
             tc.tile_pool(name="outp", bufs=1) as outp, \
             tc.tile_pool(name="popool", bufs=2) as popool:

            # ---------- inputs: 4 DMA queues, partition-major layouts ----------
            warm = consts.tile([128, 512], BF)
            nc.vector.memset(warm[:], 0.0)
            scr = consts.tile([1, 16], BF)
            scr2 = consts.tile([1, 16], BF)
            nc.vector.memset(scr[:], 0.0)
            xT = consts.tile([128, CT, N], BF)
            nc.sync.dma_start(xT[:], xT_e[:, :, :])
            fcon = consts.tile([128, 24], F32)
            nc.sync.dma_start(fcon[:], fcon_e[:, :])
            wqkA = consts.tile([128, CT, 2 * DIM], BF)
            nc.gpsimd.dma_start(wqkA[:], wqkA_e[:, :, :])
            wvB = consts.tile([128, CT, DIM], BF)
            nc.scalar.dma_start(wvB[:], wvB_e[:, :, :])
            wproj = consts.tile([128, CT, DIM], BF)
            nc.scalar.dma_start(wproj[:], wproj_e[:, :, :])

            def qkbias(ot):
                return fcon[:, ot:ot + 1]

            def pbias(ot):
                return fcon[:, 8 + ot:9 + ot]

            def maskb(mt):
                return fcon[:, 12 + mt:13 + mt]

            outT = outp.tile([128, CT, N], BF)
            v_sb = work.tile([128, NT, H, D + 1], BF)
            qk = qkpool.tile([128, OT, N], BF)
            nc.vector.memset(v_sb[:, :, :, D:D + 1], 1.0)
            # early exp-table load on a scratch tile (overlaps the DMA wait)
            nc.scalar.activation(scr2[:], scr[:], AF.Exp, bias=0.0, scale=1.0)

            # ---------- warm-up matmuls (HAM un-throttle during DMA wait) ----------
            wscr = consts.tile([1, 16], F32)
            with tc.tile_pool(name="wps", bufs=1, space="PSUM") as wps:
                wp = wps.tile([128, 512], F32)
                for i in range(8):
                    nc.tensor.matmul(wp[:, :], lhsT=warm[:, 0:128], rhs=warm[:, :],
                                     start=(i == 0), stop=(i == 7))
                # reader so later pools' bank reuse orders after the warm-up
                nc.vector.tensor_copy(wscr[0:1, :], wp[0:1, 0:16])

            with tc.tile_pool(name="lpp", bufs=2, space="PSUM") as lpp, \
                 tc.tile_pool(name="avp", bufs=2, space="PSUM") as avp:

                def qk_items(ot):
                    items = []
                    for ncx in range(NC_CH):
                        box = {}

                        def mk(ct, box=box, ncx=ncx, ot=ot):
                            def mm():
                                if "t" not in box:
                                    box["t"] = avp.tile([128, 512], F32, tag="av",
                                                        name=f"qp{ot}_{ncx}")
                                nc.tensor.matmul(
                                    box["t"][:, :],
                                    lhsT=wqkA[:, ct, ot * 128:(ot + 1) * 128],
                                    rhs=xT[:, ct, ncx * 512:(ncx + 1) * 512],
                                    start=(ct == 0), stop=(ct == CT - 1))
                            return mm

                        def ev(box=box, ncx=ncx, ot=ot):
                            nc.vector.tensor_scalar_add(
                                qk[:, ot, ncx * 512:(ncx + 1) * 512],
                                box["t"][:, :], qkbias(ot))
                        for ct in range(CT):
                            items.append(mk(ct))
                        items.append(ev)
                    return items

                # qk pair-0 weights first so pair 0 can start immediately
                for it in qk_items(0) + qk_items(4):
                    it()

                def v_items():
                    items = []
                    for mt in range(NT):
                        box = {}

                        def mk(ct, box=box, mt=mt):
                            def mm():
                                if "t" not in box:
                                    box["t"] = avp.tile([128, 512], F32, tag="av",
                                                        name=f"vp{mt}")
                                nc.tensor.matmul(
                                    box["t"][:, :],
                                    lhsT=xT[:, ct, mt * 128:(mt + 1) * 128],
                                    rhs=wvB[:, ct, :],
                                    start=(ct == 0), stop=(ct == CT - 1))
                            return mm

                        def ev(box=box, mt=mt):
                            nc.vector.tensor_copy(
                                v_sb[:, mt, :, 0:D],
                                box["t"][:, :].rearrange("p (h d) -> p h d", h=H))
                        for ct in range(CT):
                            items.append(mk(ct))
                        items.append(ev)
                    return items

                # ---------- attention ----------
                st_t = [None, None]

                def av_group(pair, hl, ncx):
                    # attn@v accumulation + denominator chain for one head/ncx
                    items = []
                    stp = st_t[pair % 2]
                    h = 2 * pair + hl
                    box = {}

                    def mk(mt, box=box, hl=hl, ncx=ncx, h=h, pair=pair):
                        def mm():
                            if "t" not in box:
                                box["t"] = avp.tile([128, 512], F32, tag="av",
                                                    name=f"ap{h}_{ncx}")
                            p0 = _stpos(pair, mt, hl, ncx)
                            nc.tensor.matmul(
                                box["t"][0:D + 1, :],
                                lhsT=v_sb[:, mt, h, :],
                                rhs=stp[:, p0:p0 + 512],
                                start=(mt == 0), stop=(mt == NT - 1))
                        return mm

                    def chain(box=box, hl=hl, ncx=ncx, pair=pair, h=h):
                        # custom DVE ops drop the input base partition: copy the
                        # den row to partition 0 before the reciprocal
                        dnr = dnpool.tile([1, 512], F32, tag="dnr",
                                          name=f"dnr{h}_{ncx}")
                        nc.vector.tensor_copy(dnr[0:1, :], box["t"][D:D + 1, :])
                        dn = dnpool.tile([1, 512], F32, tag="dn",
                                         name=f"dn{h}_{ncx}")
                        nc.vector.reciprocal_approx_fast(dn[0:1, :], dnr[0:1, :])
                        bc = bcpool.tile([D, 512], F32, tag="bc",
                                         name=f"bc{h}_{ncx}")
                        nc.gpsimd.partition_broadcast(bc[:, :], dn[0:1, :])
                        nc.vector.tensor_mul(
                            outT[hl * 64:hl * 64 + 64, pair,
                                 ncx * 512:(ncx + 1) * 512],
                            box["t"][0:D, :], bc[:, :])
                    for mt in range(NT):
                        items.append(mk(mt))
                    items.append(chain)
                    return items

                def proj_items(ncx):
                    items = []
                    nsl = slice(ncx * 512, (ncx + 1) * 512)
                    for ot in range(CT):
                        box = {}

                        def mk(kt, box=box, ot=ot, ncx=ncx):
                            def mm():
                                if "t" not in box:
                                    box["t"] = lpp.tile([128, LPW], F32, tag="lp",
                                                        name=f"pp{ot % 2}")
                                nc.tensor.matmul(
                                    box["t"][:, 0:512],
                                    lhsT=wproj[:, kt, ot * 128:(ot + 1) * 128],
                                    rhs=outT[:, kt, ncx * 512:(ncx + 1) * 512],
                                    start=(kt == 0), stop=(kt == CT - 1))
                            return mm

                        def ev(box=box, ot=ot, ncx=ncx, nsl=nsl):
                            po = popool.tile([128, 512], F32, tag="po")
                            nc.vector.tensor_scalar_add(po[:, :], box["t"][:, 0:512],
                                                        pbias(ot))
                            deng = nc.sync if (ot + ncx) % 2 == 0 else nc.gpsimd
                            deng.dma_start(out_e[ot * 128:(ot + 1) * 128, nsl],
                                           po[:, :])
                        for kt in range(CT):
                            items.append(mk(kt))
                        items.append(ev)
                    return items

                filler = []
                fi = 0

                def consume(k):
                    nonlocal fi
                    e = min(fi + k, len(filler))
                    while fi < e:
                        filler[fi]()
                        fi += 1

                for pair in range(4):
                    stp = stpool.tile([128, STW], BF, tag="st", name=f"st{pair % 2}")
                    st_t[pair % 2] = stp
                    if pair == 0:
                        filler = v_items()
                        for ot in (1, 5, 2, 6, 3, 7):
                            filler += qk_items(ot)
                        fi = 0
                    elif pair == 3:
                        # av(2) plus the ncx0 half of av(3) (ncx-major st layout
                        # means its chunks complete in the first half of pair 3)
                        rem = filler[fi:]
                        filler = rem + av_group(3, 0, 0) + av_group(3, 1, 0)
                        fi = 0
                    npts = len(_slots(pair)) // 2
                    per = max(1, (len(filler) - fi + npts - 1) // npts)

                    lpt = {}
                    acted = 0
                    for si, (mt, hl, ncx) in enumerate(_slots(pair)):
                        pos = _stpos(pair, mt, hl, ncx)
                        ti = pos // LPW
                        off = pos - ti * LPW
                        if ti not in lpt:
                            lpt[ti] = lpp.tile([128, LPW], F32, tag="lp",
                                               name=f"lp{pair}_{ti % 2}")
                        pb = hl * 64
                        nc.tensor.matmul(
                            lpt[ti][:, off:off + 512],
                            lhsT=qk[pb:pb + 64, 4 + pair, mt * 128:(mt + 1) * 128],
                            rhs=qk[pb:pb + 64, pair, ncx * 512:(ncx + 1) * 512],
                            start=True, stop=True, tile_position=(pb, 0))
                        if not mask_ones:
                            nc.scalar.activation(
                                stp[:, pos:pos + 512], lpt[ti][:, off:off + 512],
                                AF.Exp, bias=maskb(mt), scale=SCALE)
                            if off + 512 == LPW or pos + 512 == STW:
                                del lpt[ti]
                        else:
                            while acted * LPW + LPW <= pos + 512 or \
                                    pos + 512 == STW:
                                w = min(LPW, STW - acted * LPW)
                                nc.scalar.activation(
                                    stp[:, acted * LPW:acted * LPW + w],
                                    lpt[acted][:, 0:w], AF.Exp,
                                    bias=0.0, scale=SCALE)
                                del lpt[acted]
                                acted += 1
                                if acted * LPW >= STW:
                                    break
                        if si % 2 == 1:
                            consume(per)   # only between hl pairs (PE row pairing)

                    consume(len(filler))
                    if pair < 3:
                        filler = []
                        for ncx in range(NC_CH):
                            for hl in range(2):
                                filler += av_group(pair, hl, ncx)
                        fi = 0

                # ---------- tail: av(3) ncx1 + proj ----------
                filler = av_group(3, 0, 1) + av_group(3, 1, 1)
                fi = 0
                tail = proj_items(0)
                ti2 = 0
                for it in filler:
                    it()
                    # interleave proj(ncx0) (independent of av(3) ncx1)
                    if ti2 < len(tail):
                        tail[ti2]()
                        ti2 += 1
                while ti2 < len(tail):
                    tail[ti2]()
                    ti2 += 1
                for it in proj_items(1):
                    it()

    nc.compile()
    return nc


def _prep(inputs):
    x = np.asarray(inputs["x"])
    mask = np.asarray(inputs["mask"])
    qkv_w = np.asarray(inputs["qkv_w"], np.float64)
    CP_U_w = np.asarray(inputs["CP_U_w"], np.float64)
    CP_U_b = np.asarray(inputs["CP_U_b"], np.float64)
    CP_V_w = np.asarray(inputs["CP_V_w"], np.float64)
    CP_V_b = np.asarray(inputs["CP_V_b"], np.float64)
    CP_C = np.asarray(inputs["CP_C"], np.float64)
    CP_att = np.asarray(inputs["CP_attention"], np.float64)
    proj_w = np.asarray(inputs["proj_w"], np.float64)
    proj_b = np.asarray(inputs["proj_b"], np.float64)

    # fold the CP branch (affine in its input) into the dense weights
    F = np.einsum('ijr,rf->fij', CP_C, CP_att)          # (4, R, R)
    UT = CP_U_w.T                                        # (DIM, R)
    VT = CP_V_w.T                                        # (R, DIM)
    A = np.stack([UT @ F[f] @ VT for f in range(4)])     # (4, DIM, DIM)
    c = np.stack([CP_U_b @ F[f] @ VT + CP_V_b for f in range(4)])  # (4, DIM)

    Wqkv = qkv_w.T + np.concatenate([A[0], A[1], A[2]], axis=1)   # (DIM, 3*DIM)
    Wp = proj_w.T + A[3]                                          # (DIM, DIM)
    b_qk = np.concatenate([c[0], c[1]])                           # (2*DIM,)
    b_out = proj_b + c[3] + c[2] @ Wp                             # (DIM,)

    fcon = np.zeros((128, 24), np.float32)
    fcon[:, 0:8] = b_qk.reshape(OT, 128).T
    fcon[:, 8:12] = b_out.reshape(CT, 128).T

    def pmajor(w):
        # (DIM, W) -> (128, CT, W): partition p holds rows {p, 128+p, ...}
        return np.ascontiguousarray(
            w.reshape(CT, 128, w.shape[1]).transpose(1, 0, 2))

    mask_ones = bool(mask.all())
    com = {
        "wqkA": pmajor(Wqkv[:, 0:2 * DIM]).astype(bf16),
        "wvB": pmajor(Wqkv[:, 2 * DIM:3 * DIM]).astype(bf16),
        "wproj": pmajor(Wp).astype(bf16),
    }
    in_maps = []
    for b in range(B):
        m = dict(com)
        m["xT"] = pmajor(x[b].T.astype(np.float64)).astype(bf16)
        fc = fcon.copy()
        if not mask_ones:
            mb = np.where(mask[b], 0.0, -1e30).astype(np.float32)
            fc[:, 12:20] = mb.reshape(NT, 128).T
        m["fcon"] = fc
        in_maps.append(m)
    return in_maps, mask_ones


LAST_EXEC_NS = None


def kernel(**inputs):
    global LAST_EXEC_NS
    in_maps, mask_ones = _prep(inputs)
    key = ("nc", mask_ones)
    if key not in _CACHE:
        _CACHE[key] = _build(mask_ones)
    nc = _CACHE[key]
    res = run_bass_kernel_spmd(nc, in_maps, core_ids=list(range(NCORES)))
    LAST_EXEC_NS = res.exec_time_ns
    out = np.stack([np.ascontiguousarray(res.results[i]["out"].T)
                    for i in range(NCORES)])
    return out.astype(np.float32)


# revision 16
# speedup vs baseline: 1.2545x; 1.0040x over previous
"""Trainium2 Bass kernel for nn_CP_Attention_Action (dense transformer block with
CP-factored low-rank corrections).

Data-parallel over batch B=8 -> one batch per NeuronCore, no collectives.

The CP branch is affine in its input, so it is folded into the dense weights on
the host (f64): W_qkv_eff = qkv_w.T + U.T @ F_f @ V.T blocks, with the q/k
biases applied at evacuation and the v bias folded through softmax (weights sum
to 1) into the proj bias. The device then runs a plain attention block:

  qkT (o,n) via stationary W tiles; v in natural (m,d) layout with a ones
  column so the softmax denominator falls out of attn@v; logits via 64x128
  row-group-paired matmuls (hl0/hl1 issued adjacently -> both stream on PE
  concurrently); exp on ScalarE in 1536-wide batches (all-ones mask -> uniform
  zero bias); denominator: copy+reciprocal on DVE + partition_broadcast on
  GpSimd; attn@v of pair p runs as PE filler during pair p+1; pair 3 uses an
  ncx-major st layout so its ncx0 attn@v half completes mid-pair and only the
  ncx1 half + proj remain in the tail. Host pre-arranges all inputs into
  partition-major layout for fat contiguous DMA descriptors on 4 queues.
"""

import os
import numpy as np
import ml_dtypes

from concourse import bacc
import concourse.mybir as mybir
import concourse.tile as tile
from concourse.bass_utils import run_bass_kernel_spmd

B, N, DIM = 8, 1024, 512
H, D = 8, 64
SCALE = D ** -0.5
NCORES = 8
NC_CH = 2          # n chunks of 512
NT = N // 128      # 8 key tiles
CT = DIM // 128    # 4 c-tiles
OT = 8             # q,k o-tiles
PAIRW = 2 * N      # free elems per key-tile slot in st (2 heads x 1024 q)
STW = NT * PAIRW   # st elems per pair (16384)
LPW = 1536         # activation batch width (3 PSUM banks)
F32 = mybir.dt.float32
BF = mybir.dt.bfloat16
AF = mybir.ActivationFunctionType
bf16 = ml_dtypes.bfloat16

_CACHE = {}


QK_ORDER = (0, 4, 1, 5, 2, 6, 3, 7)


def _stpos(pair, mt, hl, ncx):
    # ncx-major: the first half of each pair's stream only needs the first
    # n-half of q, so pair 0 can start before the second xT half lands
    return ncx * (NT * 1024) + mt * 1024 + hl * 512


def _slots(pair):
    # (mt, hl, ncx) in ascending stream-position order, hl adjacent
    out = []
    for ncx in range(NC_CH):
        for mt in range(NT):
            for hl in range(2):
                out.append((mt, hl, ncx))
    return out


def _build(mask_ones: bool):
    nc = bacc.Bacc()

    xA_e = nc.declare_dram_parameter("xA", [128, CT, 512], BF, isOutput=False)
    xB_e = nc.declare_dram_parameter("xB", [128, CT, 512], BF, isOutput=False)
    wqk1_e = nc.declare_dram_parameter("wqk1", [128, CT, 256], BF, isOutput=False)
    wqk2_e = nc.declare_dram_parameter("wqk2", [128, CT, 768], BF, isOutput=False)
    wvB_e = nc.declare_dram_parameter("wvB", [128, CT, DIM], BF, isOutput=False)
    wproj_e = nc.declare_dram_parameter("wproj", [128, CT, DIM], BF, isOutput=False)
    fcon_e = nc.declare_dram_parameter("fcon", [128, 24], F32, isOutput=False)
    out_e = nc.declare_dram_parameter("out", [DIM, N], F32, isOutput=True)

    # fcon layout (f32 columns): 0:8 qk bias per ot | 8:12 proj bias | 12:20 maskb
    with tile.TileContext(nc) as tc:
        with tc.tile_pool(name="consts", bufs=1) as consts, \
             tc.tile_pool(name="qkpool", bufs=1) as qkpool, \
             tc.tile_pool(name="stpool", bufs=2) as stpool, \
             tc.tile_pool(name="work", bufs=1) as work, \
             tc.tile_pool(name="dnpool", bufs=4) as dnpool, \
             tc.tile_pool(name="bcpool", bufs=2) as bcpool, \
             tc.tile_pool(name="outp", bufs=1) as outp, \
             tc.tile_pool(name="popool", bufs=2) as popool:

            # ---------- inputs: 4 DMA queues, partition-major layouts ----------
            warm = consts.tile([128, 512], BF)
            nc.vector.memset(warm[:], 0.0)
            scr = consts.tile([1, 16], BF)
            scr2 = consts.tile([1, 16], BF)
            nc.vector.memset(scr[:], 0.0)
            xh = [consts.tile([128, CT, 512], BF, name=f"xh{i}") for i in range(2)]
            nc.sync.dma_start(xh[0][:], xA_e[:, :, :])
            nc.sync.dma_start(xh[1][:], xB_e[:, :, :])
            fcon = consts.tile([128, 24], F32)
            nc.sync.dma_start(fcon[:], fcon_e[:, :])
            wqk1 = consts.tile([128, CT, 256], BF)
            nc.gpsimd.dma_start(wqk1[:], wqk1_e[:, :, :])
            wqk2 = consts.tile([128, CT, 768], BF)
            nc.gpsimd.dma_start(wqk2[:], wqk2_e[:, :, :])
            wvB = consts.tile([128, CT, DIM], BF)
            nc.scalar.dma_start(wvB[:], wvB_e[:, :, :])
            wproj = consts.tile([128, CT, DIM], BF)
            nc.scalar.dma_start(wproj[:], wproj_e[:, :, :])

            def qkbias(ot):
                return fcon[:, ot:ot + 1]

            def pbias(ot):
                return fcon[:, 8 + ot:9 + ot]

            def maskb(mt):
                return fcon[:, 12 + mt:13 + mt]

            outT = outp.tile([128, CT, N], BF)
            v_sb = work.tile([128, NT, H, D + 1], BF)
            qk = qkpool.tile([128, OT, N], BF)
            nc.vector.memset(v_sb[:, :, :, D:D + 1], 1.0)
            # early exp-table load on a scratch tile (overlaps the DMA wait)
            nc.scalar.activation(scr2[:], scr[:], AF.Exp, bias=0.0, scale=1.0)

            # ---------- warm-up matmuls (HAM un-throttle during DMA wait) ----------
            wscr = consts.tile([1, 16], F32)
            with tc.tile_pool(name="wps", bufs=1, space="PSUM") as wps:
                wp = wps.tile([128, 512], F32)
                for i in range(4):
                    nc.tensor.matmul(wp[:, :], lhsT=warm[:, 0:128], rhs=warm[:, :],
                                     start=(i == 0), stop=(i == 3))
                # reader so later pools' bank reuse orders after the warm-up
                nc.vector.tensor_copy(wscr[0:1, :], wp[0:1, 0:16])

            with tc.tile_pool(name="lpp", bufs=2, space="PSUM") as lpp, \
                 tc.tile_pool(name="avp", bufs=2, space="PSUM") as avp:

                def qk_items(ot):
                    items = []
                    for ncx in range(NC_CH):
                        box = {}

                        def mk(ct, box=box, ncx=ncx, ot=ot):
                            def mm():
                                if "t" not in box:
                                    box["t"] = avp.tile([128, 512], F32, tag="av",
                                                        name=f"qp{ot}_{ncx}")
                                j = QK_ORDER.index(ot)
                                w = wqk1[:, ct, j * 128:(j + 1) * 128] if j < 2 \
                                    else wqk2[:, ct, (j - 2) * 128:(j - 1) * 128]
                                nc.tensor.matmul(
                                    box["t"][:, :], lhsT=w,
                                    rhs=xh[ncx][:, ct, :],
                                    start=(ct == 0), stop=(ct == CT - 1))
                            return mm

                        def ev(box=box, ncx=ncx, ot=ot):
                            nc.vector.tensor_scalar_add(
                                qk[:, ot, ncx * 512:(ncx + 1) * 512],
                                box["t"][:, :], qkbias(ot))
                        for ct in range(CT):
                            items.append(mk(ct))
                        items.append(ev)
                    return items

                # qk pair-0 weights first so pair 0 can start immediately
                for it in qk_items(0) + qk_items(4):
                    it()

                def v_items():
                    items = []
                    for mt in range(NT):
                        box = {}

                        def mk(ct, box=box, mt=mt):
                            def mm():
                                if "t" not in box:
                                    box["t"] = avp.tile([128, 512], F32, tag="av",
                                                        name=f"vp{mt}")
                                nc.tensor.matmul(
                                    box["t"][:, :],
                                    lhsT=xh[mt // 4][:, ct, (mt % 4) * 128:
                                                     (mt % 4) * 128 + 128],
                                    rhs=wvB[:, ct, :],
                                    start=(ct == 0), stop=(ct == CT - 1))
                            return mm

                        def ev(box=box, mt=mt):
                            nc.vector.tensor_copy(
                                v_sb[:, mt, :, 0:D],
                                box["t"][:, :].rearrange("p (h d) -> p h d", h=H))
                        for ct in range(CT):
                            items.append(mk(ct))
                        items.append(ev)
                    return items

                # ---------- attention ----------
                st_t = [None, None]

                def av_group(pair, hl, ncx):
                    # attn@v accumulation + denominator chain for one head/ncx
                    items = []
                    stp = st_t[pair % 2]
                    h = 2 * pair + hl
                    box = {}

                    def mk(mt, box=box, hl=hl, ncx=ncx, h=h, pair=pair):
                        def mm():
                            if "t" not in box:
                                box["t"] = avp.tile([128, 512], F32, tag="av",
                                                    name=f"ap{h}_{ncx}")
                            p0 = _stpos(pair, mt, hl, ncx)
                            nc.tensor.matmul(
                                box["t"][0:D + 1, :],
                                lhsT=v_sb[:, mt, h, :],
                                rhs=stp[:, p0:p0 + 512],
                                start=(mt == 0), stop=(mt == NT - 1))
                        return mm

                    def chain(box=box, hl=hl, ncx=ncx, pair=pair, h=h):
                        # custom DVE ops drop the input base partition: copy the
                        # den row to partition 0 before the reciprocal
                        dnr = dnpool.tile([1, 512], F32, tag="dnr",
                                          name=f"dnr{h}_{ncx}")
                        nc.vector.tensor_copy(dnr[0:1, :], box["t"][D:D + 1, :])
                        dn = dnpool.tile([1, 512], F32, tag="dn",
                                         name=f"dn{h}_{ncx}")
                        nc.vector.reciprocal_approx_fast(dn[0:1, :], dnr[0:1, :])
                        bc = bcpool.tile([D, 512], F32, tag="bc",
                                         name=f"bc{h}_{ncx}")
                        nc.gpsimd.partition_broadcast(bc[:, :], dn[0:1, :])
                        nc.vector.tensor_mul(
                            outT[hl * 64:hl * 64 + 64, pair,
                                 ncx * 512:(ncx + 1) * 512],
                            box["t"][0:D, :], bc[:, :])
                    for mt in range(NT):
                        items.append(mk(mt))
                    items.append(chain)
                    return items

                def proj_items(ncx):
                    items = []
                    nsl = slice(ncx * 512, (ncx + 1) * 512)
                    for ot in range(CT):
                        box = {}

                        def mk(kt, box=box, ot=ot, ncx=ncx):
                            def mm():
                                if "t" not in box:
                                    box["t"] = lpp.tile([128, LPW], F32, tag="lp",
                                                        name=f"pp{ot % 2}")
                                nc.tensor.matmul(
                                    box["t"][:, 0:512],
                                    lhsT=wproj[:, kt, ot * 128:(ot + 1) * 128],
                                    rhs=outT[:, kt, ncx * 512:(ncx + 1) * 512],
                                    start=(kt == 0), stop=(kt == CT - 1))
                            return mm

                        def ev(box=box, ot=ot, ncx=ncx, nsl=nsl):
                            po = popool.tile([128, 512], F32, tag="po")
                            nc.vector.tensor_scalar_add(po[:, :], box["t"][:, 0:512],
                                                        pbias(ot))
                            deng = nc.sync if (ot + ncx) % 2 == 0 else nc.gpsimd
                            deng.dma_start(out_e[ot * 128:(ot + 1) * 128, nsl],
                                           po[:, :])
                        for kt in range(CT):
                            items.append(mk(kt))
                        items.append(ev)
                    return items

                filler = []
                fi = 0

                def consume(k):
                    nonlocal fi
                    e = min(fi + k, len(filler))
                    while fi < e:
                        filler[fi]()
                        fi += 1

                for pair in range(4):
                    stp = stpool.tile([128, STW], BF, tag="st", name=f"st{pair % 2}")
                    st_t[pair % 2] = stp
                    if pair == 0:
                        filler = v_items()
                        for ot in (1, 5, 2, 6):
                            filler += qk_items(ot)
                        fi = 0
                    elif pair == 1:
                        # av(0) (set at end of pair 0) + held-back qk
                        filler = filler[fi:] + qk_items(3) + qk_items(7)
                        fi = 0
                    elif pair == 3:
                        # av(2) plus the ncx0 half of av(3) (ncx-major st layout
                        # means its chunks complete in the first half of pair 3)
                        rem = filler[fi:]
                        filler = rem + av_group(3, 0, 0) + av_group(3, 1, 0)
                        fi = 0
                    npts = len(_slots(pair)) // 2
                    per = max(1, (len(filler) - fi + npts - 1) // npts)

                    lpt = {}
                    acted = 0
                    for si, (mt, hl, ncx) in enumerate(_slots(pair)):
                        pos = _stpos(pair, mt, hl, ncx)
                        ti = pos // LPW
                        off = pos - ti * LPW
                        if ti not in lpt:
                            lpt[ti] = lpp.tile([128, LPW], F32, tag="lp",
                                               name=f"lp{pair}_{ti % 2}")
                        pb = hl * 64
                        nc.tensor.matmul(
                            lpt[ti][:, off:off + 512],
                            lhsT=qk[pb:pb + 64, 4 + pair, mt * 128:(mt + 1) * 128],
                            rhs=qk[pb:pb + 64, pair, ncx * 512:(ncx + 1) * 512],
                            start=True, stop=True, tile_position=(pb, 0))
                        if not mask_ones:
                            nc.scalar.activation(
                                stp[:, pos:pos + 512], lpt[ti][:, off:off + 512],
                                AF.Exp, bias=maskb(mt), scale=SCALE)
                            if off + 512 == LPW or pos + 512 == STW:
                                del lpt[ti]
                        else:
                            while acted * LPW + LPW <= pos + 512 or \
                                    pos + 512 == STW:
                                w = min(LPW, STW - acted * LPW)
                                nc.scalar.activation(
                                    stp[:, acted * LPW:acted * LPW + w],
                                    lpt[acted][:, 0:w], AF.Exp,
                                    bias=0.0, scale=SCALE)
                                del lpt[acted]
                                acted += 1
                                if acted * LPW >= STW:
                                    break
                        if si % 2 == 1:
                            consume(per)   # only between hl pairs (PE row pairing)

                    consume(len(filler))
                    if pair < 3:
                        filler = []
                        for ncx in range(NC_CH):
                            for hl in range(2):
                                filler += av_group(pair, hl, ncx)
                        fi = 0

                # ---------- tail: av(3) ncx1 + proj ----------
                filler = av_group(3, 0, 1) + av_group(3, 1, 1)
                fi = 0
                tail = proj_items(0)
                ti2 = 0
                for it in filler:
                    it()
                    # interleave proj(ncx0) (independent of av(3) ncx1)
                    if ti2 < len(tail):
                        tail[ti2]()
                        ti2 += 1
                while ti2 < len(tail):
                    tail[ti2]()
                    ti2 += 1
                for it in proj_items(1):
                    it()

    nc.compile()
    return nc


def _prep(inputs):
    x = np.asarray(inputs["x"])
    mask = np.asarray(inputs["mask"])
    qkv_w = np.asarray(inputs["qkv_w"], np.float64)
    CP_U_w = np.asarray(inputs["CP_U_w"], np.float64)
    CP_U_b = np.asarray(inputs["CP_U_b"], np.float64)
    CP_V_w = np.asarray(inputs["CP_V_w"], np.float64)
    CP_V_b = np.asarray(inputs["CP_V_b"], np.float64)
    CP_C = np.asarray(inputs["CP_C"], np.float64)
    CP_att = np.asarray(inputs["CP_attention"], np.float64)
    proj_w = np.asarray(inputs["proj_w"], np.float64)
    proj_b = np.asarray(inputs["proj_b"], np.float64)

    # fold the CP branch (affine in its input) into the dense weights
    F = np.einsum('ijr,rf->fij', CP_C, CP_att)          # (4, R, R)
    UT = CP_U_w.T                                        # (DIM, R)
    VT = CP_V_w.T                                        # (R, DIM)
    A = np.stack([UT @ F[f] @ VT for f in range(4)])     # (4, DIM, DIM)
    c = np.stack([CP_U_b @ F[f] @ VT + CP_V_b for f in range(4)])  # (4, DIM)

    Wqkv = qkv_w.T + np.concatenate([A[0], A[1], A[2]], axis=1)   # (DIM, 3*DIM)
    Wp = proj_w.T + A[3]                                          # (DIM, DIM)
    b_qk = np.concatenate([c[0], c[1]])                           # (2*DIM,)
    b_out = proj_b + c[3] + c[2] @ Wp                             # (DIM,)

    fcon = np.zeros((128, 24), np.float32)
    fcon[:, 0:8] = b_qk.reshape(OT, 128).T
    fcon[:, 8:12] = b_out.reshape(CT, 128).T

    def pmajor(w):
        # (DIM, W) -> (128, CT, W): partition p holds rows {p, 128+p, ...}
        return np.ascontiguousarray(
            w.reshape(CT, 128, w.shape[1]).transpose(1, 0, 2))

    mask_ones = bool(mask.all())
    wqk = pmajor(Wqkv[:, 0:2 * DIM])            # (128, CT, 1024)
    wqk_ord = np.concatenate([wqk[:, :, ot * 128:(ot + 1) * 128]
                              for ot in QK_ORDER], axis=2)
    com = {
        "wqk1": np.ascontiguousarray(wqk_ord[:, :, 0:256]).astype(bf16),
        "wqk2": np.ascontiguousarray(wqk_ord[:, :, 256:1024]).astype(bf16),
        "wvB": pmajor(Wqkv[:, 2 * DIM:3 * DIM]).astype(bf16),
        "wproj": pmajor(Wp).astype(bf16),
    }
    in_maps = []
    for b in range(B):
        m = dict(com)
        xp = pmajor(x[b].T.astype(np.float64))
        m["xA"] = np.ascontiguousarray(xp[:, :, 0:512]).astype(bf16)
        m["xB"] = np.ascontiguousarray(xp[:, :, 512:1024]).astype(bf16)
        fc = fcon.copy()
        if not mask_ones:
            mb = np.where(mask[b], 0.0, -1e30).astype(np.float32)
            fc[:, 12:20] = mb.reshape(NT, 128).T
        m["fcon"] = fc
        in_maps.append(m)
    return in_maps, mask_ones


LAST_EXEC_NS = None


def kernel(**inputs):
    global LAST_EXEC_NS
    in_maps, mask_ones = _prep(inputs)
    key = ("nc", mask_ones)
    if key not in _CACHE:
        _CACHE[key] = _build(mask_ones)
    nc = _CACHE[key]
    res = run_bass_kernel_spmd(nc, in_maps, core_ids=list(range(NCORES)))
    LAST_EXEC_NS = res.exec_time_ns
    out = np.stack([np.ascontiguousarray(res.results[i]["out"].T)
                    for i in range(NCORES)])
    return out.astype(np.float32)


# revision 17
# speedup vs baseline: 1.2848x; 1.0241x over previous
"""Trainium2 Bass kernel for nn_CP_Attention_Action (dense transformer block with
CP-factored low-rank corrections).

Data-parallel over batch B=8 -> one batch per NeuronCore, no collectives.

The CP branch is affine in its input, so it is folded into the dense weights on
the host (f64): W_qkv_eff = qkv_w.T + U.T @ F_f @ V.T blocks, with the q/k
biases applied at evacuation and the v bias folded through softmax (weights sum
to 1) into the proj bias. The device then runs a plain attention block:

  qkT (o,n) via stationary W tiles; v in natural (m,d) layout with a ones
  column so the softmax denominator falls out of attn@v; logits via 64x128
  row-group-paired matmuls (hl0/hl1 issued adjacently -> both stream on PE
  concurrently); exp on ScalarE in 1536-wide batches (all-ones mask -> uniform
  zero bias); denominator: copy+reciprocal on DVE + partition_broadcast on
  GpSimd; attn@v of pair p runs as PE filler during pair p+1; pair 3 uses an
  ncx-major st layout so its ncx0 attn@v half completes mid-pair and only the
  ncx1 half + proj remain in the tail. Host pre-arranges all inputs into
  partition-major layout for fat contiguous DMA descriptors on 4 queues.
"""

import os
import numpy as np
import ml_dtypes

from concourse import bacc
import concourse.mybir as mybir
import concourse.tile as tile
from concourse.bass_utils import run_bass_kernel_spmd

B, N, DIM = 8, 1024, 512
H, D = 8, 64
SCALE = D ** -0.5
NCORES = 8
NC_CH = 2          # n chunks of 512
NT = N // 128      # 8 key tiles
CT = DIM // 128    # 4 c-tiles
OT = 8             # q,k o-tiles
PAIRW = 2 * N      # free elems per key-tile slot in st (2 heads x 1024 q)
STW = NT * PAIRW   # st elems per pair (16384)
LPW = 1536         # activation batch width (3 PSUM banks)
F32 = mybir.dt.float32
BF = mybir.dt.bfloat16
AF = mybir.ActivationFunctionType
bf16 = ml_dtypes.bfloat16

_CACHE = {}


QK_ORDER = (0, 4, 1, 5, 2, 6, 3, 7)


def _stpos(pair, mt, hl, ncx):
    # ncx-major: the first half of each pair's stream only needs the first
    # n-half of q, so pair 0 can start before the second xT half lands
    return ncx * (NT * 1024) + mt * 1024 + hl * 512


def _slots(pair):
    # (mt, hl, ncx) in ascending stream-position order, hl adjacent
    out = []
    for ncx in range(NC_CH):
        for mt in range(NT):
            for hl in range(2):
                out.append((mt, hl, ncx))
    return out


def _build(mask_ones: bool):
    nc = bacc.Bacc()

    xA_e = nc.declare_dram_parameter("xA", [128, CT, 512], BF, isOutput=False)
    xB_e = nc.declare_dram_parameter("xB", [128, CT, 512], BF, isOutput=False)
    wqk1_e = nc.declare_dram_parameter("wqk1", [128, CT, 256], BF, isOutput=False)
    wqk2_e = nc.declare_dram_parameter("wqk2", [128, CT, 768], BF, isOutput=False)
    wvB_e = nc.declare_dram_parameter("wvB", [128, CT, DIM], BF, isOutput=False)
    wproj_e = nc.declare_dram_parameter("wproj", [128, CT, DIM], BF, isOutput=False)
    fcon_e = nc.declare_dram_parameter("fcon", [128, 24], F32, isOutput=False)
    out_e = nc.declare_dram_parameter("out", [128, NC_CH, CT, 512], BF, isOutput=True)

    # fcon layout (f32 columns): 0:8 qk bias per ot | 8:12 proj bias | 12:20 maskb
    with tile.TileContext(nc) as tc:
        with tc.tile_pool(name="consts", bufs=1) as consts, \
             tc.tile_pool(name="qkpool", bufs=1) as qkpool, \
             tc.tile_pool(name="stpool", bufs=2) as stpool, \
             tc.tile_pool(name="work", bufs=1) as work, \
             tc.tile_pool(name="dnpool", bufs=4) as dnpool, \
             tc.tile_pool(name="bcpool", bufs=2) as bcpool, \
             tc.tile_pool(name="outp", bufs=1) as outp, \
             tc.tile_pool(name="popool", bufs=1) as popool:

            # ---------- inputs: 4 DMA queues, partition-major layouts ----------
            warm = consts.tile([128, 512], BF)
            nc.vector.memset(warm[:], 0.0)
            scr = consts.tile([1, 16], BF)
            scr2 = consts.tile([1, 16], BF)
            nc.vector.memset(scr[:], 0.0)
            xh = [consts.tile([128, CT, 512], BF, name=f"xh{i}") for i in range(2)]
            fcon = consts.tile([128, 24], F32)
            wqk1 = consts.tile([128, CT, 256], BF)
            wqk2 = consts.tile([128, CT, 768], BF)
            nc.sync.dma_start(xh[0][:], xA_e[:, :, :])
            nc.sync.dma_start(wqk1[:], wqk1_e[:, :, :])
            nc.sync.dma_start(fcon[:], fcon_e[:, :])
            nc.sync.dma_start(xh[1][:], xB_e[:, :, :])
            nc.gpsimd.dma_start(wqk2[:], wqk2_e[:, :, :])
            wvB = consts.tile([128, CT, DIM], BF)
            nc.scalar.dma_start(wvB[:], wvB_e[:, :, :])
            wproj = consts.tile([128, CT, DIM], BF)
            nc.scalar.dma_start(wproj[:], wproj_e[:, :, :])

            def qkbias(ot):
                return fcon[:, ot:ot + 1]

            def pbias(ot):
                return fcon[:, 8 + ot:9 + ot]

            def maskb(mt):
                return fcon[:, 12 + mt:13 + mt]

            outT = outp.tile([128, CT, N], BF)
            v_sb = work.tile([128, NT, H, D + 1], BF)
            qk = qkpool.tile([128, OT, N], BF)
            nc.vector.memset(v_sb[:, :, :, D:D + 1], 1.0)
            # early exp-table load on a scratch tile (overlaps the DMA wait)
            nc.scalar.activation(scr2[:], scr[:], AF.Exp, bias=0.0, scale=1.0)

            # ---------- warm-up matmuls (HAM un-throttle during DMA wait) ----------
            wscr = consts.tile([1, 16], F32)
            with tc.tile_pool(name="wps", bufs=1, space="PSUM") as wps:
                wp = wps.tile([128, 512], F32)
                for i in range(4):
                    nc.tensor.matmul(wp[:, :], lhsT=warm[:, 0:128], rhs=warm[:, :],
                                     start=(i == 0), stop=(i == 3))
                # reader so later pools' bank reuse orders after the warm-up
                nc.vector.tensor_copy(wscr[0:1, :], wp[0:1, 0:16])

            with tc.tile_pool(name="lpp", bufs=2, space="PSUM") as lpp, \
                 tc.tile_pool(name="avp", bufs=2, space="PSUM") as avp:

                def qk_items(ot):
                    items = []
                    for ncx in range(NC_CH):
                        box = {}

                        def mk(ct, box=box, ncx=ncx, ot=ot):
                            def mm():
                                if "t" not in box:
                                    box["t"] = avp.tile([128, 512], F32, tag="av",
                                                        name=f"qp{ot}_{ncx}")
                                j = QK_ORDER.index(ot)
                                w = wqk1[:, ct, j * 128:(j + 1) * 128] if j < 2 \
                                    else wqk2[:, ct, (j - 2) * 128:(j - 1) * 128]
                                nc.tensor.matmul(
                                    box["t"][:, :], lhsT=w,
                                    rhs=xh[ncx][:, ct, :],
                                    start=(ct == 0), stop=(ct == CT - 1))
                            return mm

                        def ev(box=box, ncx=ncx, ot=ot):
                            nc.vector.tensor_scalar_add(
                                qk[:, ot, ncx * 512:(ncx + 1) * 512],
                                box["t"][:, :], qkbias(ot))
                        for ct in range(CT):
                            items.append(mk(ct))
                        items.append(ev)
                    return items

                # qk pair-0 weights first so pair 0 can start immediately
                for it in qk_items(0) + qk_items(4):
                    it()

                def v_items():
                    items = []
                    for mt in range(NT):
                        box = {}

                        def mk(ct, box=box, mt=mt):
                            def mm():
                                if "t" not in box:
                                    box["t"] = avp.tile([128, 512], F32, tag="av",
                                                        name=f"vp{mt}")
                                nc.tensor.matmul(
                                    box["t"][:, :],
                                    lhsT=xh[mt // 4][:, ct, (mt % 4) * 128:
                                                     (mt % 4) * 128 + 128],
                                    rhs=wvB[:, ct, :],
                                    start=(ct == 0), stop=(ct == CT - 1))
                            return mm

                        def ev(box=box, mt=mt):
                            nc.vector.tensor_copy(
                                v_sb[:, mt, :, 0:D],
                                box["t"][:, :].rearrange("p (h d) -> p h d", h=H))
                        for ct in range(CT):
                            items.append(mk(ct))
                        items.append(ev)
                    return items

                # ---------- attention ----------
                st_t = [None, None]

                def av_group(pair, hl, ncx):
                    # attn@v accumulation + denominator chain for one head/ncx
                    items = []
                    stp = st_t[pair % 2]
                    h = 2 * pair + hl
                    box = {}

                    def mk(mt, box=box, hl=hl, ncx=ncx, h=h, pair=pair):
                        def mm():
                            if "t" not in box:
                                box["t"] = avp.tile([128, 512], F32, tag="av",
                                                    name=f"ap{h}_{ncx}")
                            p0 = _stpos(pair, mt, hl, ncx)
                            nc.tensor.matmul(
                                box["t"][0:D + 1, :],
                                lhsT=v_sb[:, mt, h, :],
                                rhs=stp[:, p0:p0 + 512],
                                start=(mt == 0), stop=(mt == NT - 1))
                        return mm

                    def chain(box=box, hl=hl, ncx=ncx, pair=pair, h=h):
                        # custom DVE ops drop the input base partition: copy the
                        # den row to partition 0 before the reciprocal
                        dnr = dnpool.tile([1, 512], F32, tag="dnr",
                                          name=f"dnr{h}_{ncx}")
                        nc.vector.tensor_copy(dnr[0:1, :], box["t"][D:D + 1, :])
                        dn = dnpool.tile([1, 512], F32, tag="dn",
                                         name=f"dn{h}_{ncx}")
                        nc.vector.reciprocal_approx_fast(dn[0:1, :], dnr[0:1, :])
                        bc = bcpool.tile([D, 512], F32, tag="bc",
                                         name=f"bc{h}_{ncx}")
                        nc.gpsimd.partition_broadcast(bc[:, :], dn[0:1, :])
                        nc.vector.tensor_mul(
                            outT[hl * 64:hl * 64 + 64, pair,
                                 ncx * 512:(ncx + 1) * 512],
                            box["t"][0:D, :], bc[:, :])
                    for mt in range(NT):
                        items.append(mk(mt))
                    items.append(chain)
                    return items

                po_sb = popool.tile([128, NC_CH, CT, 512], BF)

                def proj_items(ncx):
                    items = []
                    for ot in range(CT):
                        box = {}

                        def mk(kt, box=box, ot=ot, ncx=ncx):
                            def mm():
                                if "t" not in box:
                                    box["t"] = lpp.tile([128, LPW], F32, tag="lp",
                                                        name=f"pp{ot % 2}")
                                nc.tensor.matmul(
                                    box["t"][:, 0:512],
                                    lhsT=wproj[:, kt, ot * 128:(ot + 1) * 128],
                                    rhs=outT[:, kt, ncx * 512:(ncx + 1) * 512],
                                    start=(kt == 0), stop=(kt == CT - 1))
                            return mm

                        def ev(box=box, ot=ot, ncx=ncx):
                            nc.vector.tensor_scalar_add(
                                po_sb[:, ncx, ot, :], box["t"][:, 0:512], pbias(ot))
                        for kt in range(CT):
                            items.append(mk(kt))
                        items.append(ev)

                    def dma(ncx=ncx):
                        deng = nc.sync if ncx == 0 else nc.gpsimd
                        deng.dma_start(out_e[:, ncx, :, :], po_sb[:, ncx, :, :])
                    items.append(dma)
                    return items

                filler = []
                fi = 0

                def consume(k):
                    nonlocal fi
                    e = min(fi + k, len(filler))
                    while fi < e:
                        filler[fi]()
                        fi += 1

                for pair in range(4):
                    stp = stpool.tile([128, STW], BF, tag="st", name=f"st{pair % 2}")
                    st_t[pair % 2] = stp
                    if pair == 0:
                        filler = v_items()
                        for ot in (1, 5, 2, 6):
                            filler += qk_items(ot)
                        fi = 0
                    elif pair == 1:
                        # av(0) (set at end of pair 0) + held-back qk
                        filler = filler[fi:] + qk_items(3) + qk_items(7)
                        fi = 0
                    elif pair == 3:
                        # av(2) plus the ncx0 half of av(3) (ncx-major st layout
                        # means its chunks complete in the first half of pair 3)
                        rem = filler[fi:]
                        filler = rem + av_group(3, 0, 0) + av_group(3, 1, 0)
                        fi = 0
                    npts = len(_slots(pair)) // 2
                    per = max(1, (len(filler) - fi + npts - 1) // npts)

                    lpt = {}
                    acted = 0
                    for si, (mt, hl, ncx) in enumerate(_slots(pair)):
                        pos = _stpos(pair, mt, hl, ncx)
                        ti = pos // LPW
                        off = pos - ti * LPW
                        if ti not in lpt:
                            lpt[ti] = lpp.tile([128, LPW], F32, tag="lp",
                                               name=f"lp{pair}_{ti % 2}")
                        pb = hl * 64
                        nc.tensor.matmul(
                            lpt[ti][:, off:off + 512],
                            lhsT=qk[pb:pb + 64, 4 + pair, mt * 128:(mt + 1) * 128],
                            rhs=qk[pb:pb + 64, pair, ncx * 512:(ncx + 1) * 512],
                            start=True, stop=True, tile_position=(pb, 0))
                        if not mask_ones:
                            nc.scalar.activation(
                                stp[:, pos:pos + 512], lpt[ti][:, off:off + 512],
                                AF.Exp, bias=maskb(mt), scale=SCALE)
                            if off + 512 == LPW or pos + 512 == STW:
                                del lpt[ti]
                        else:
                            while acted * LPW + LPW <= pos + 512 or \
                                    pos + 512 == STW:
                                w = min(LPW, STW - acted * LPW)
                                nc.scalar.activation(
                                    stp[:, acted * LPW:acted * LPW + w],
                                    lpt[acted][:, 0:w], AF.Exp,
                                    bias=0.0, scale=SCALE)
                                del lpt[acted]
                                acted += 1
                                if acted * LPW >= STW:
                                    break
                        if si % 2 == 1:
                            consume(per)   # only between hl pairs (PE row pairing)

                    consume(len(filler))
                    if pair < 3:
                        filler = []
                        for ncx in range(NC_CH):
                            for hl in range(2):
                                filler += av_group(pair, hl, ncx)
                        fi = 0

                # ---------- tail: av(3) ncx1 + proj ----------
                filler = av_group(3, 0, 1) + av_group(3, 1, 1)
                fi = 0
                tail = proj_items(0)
                ti2 = 0
                for it in filler:
                    it()
                    # interleave proj(ncx0) (independent of av(3) ncx1)
                    if ti2 < len(tail):
                        tail[ti2]()
                        ti2 += 1
                while ti2 < len(tail):
                    tail[ti2]()
                    ti2 += 1
                for it in proj_items(1):
                    it()

    nc.compile()
    return nc


def _prep(inputs):
    x = np.asarray(inputs["x"])
    mask = np.asarray(inputs["mask"])
    qkv_w = np.asarray(inputs["qkv_w"], np.float64)
    CP_U_w = np.asarray(inputs["CP_U_w"], np.float64)
    CP_U_b = np.asarray(inputs["CP_U_b"], np.float64)
    CP_V_w = np.asarray(inputs["CP_V_w"], np.float64)
    CP_V_b = np.asarray(inputs["CP_V_b"], np.float64)
    CP_C = np.asarray(inputs["CP_C"], np.float64)
    CP_att = np.asarray(inputs["CP_attention"], np.float64)
    proj_w = np.asarray(inputs["proj_w"], np.float64)
    proj_b = np.asarray(inputs["proj_b"], np.float64)

    # fold the CP branch (affine in its input) into the dense weights
    F = np.einsum('ijr,rf->fij', CP_C, CP_att)          # (4, R, R)
    UT = CP_U_w.T                                        # (DIM, R)
    VT = CP_V_w.T                                        # (R, DIM)
    A = np.stack([UT @ F[f] @ VT for f in range(4)])     # (4, DIM, DIM)
    c = np.stack([CP_U_b @ F[f] @ VT + CP_V_b for f in range(4)])  # (4, DIM)

    Wqkv = qkv_w.T + np.concatenate([A[0], A[1], A[2]], axis=1)   # (DIM, 3*DIM)
    Wp = proj_w.T + A[3]                                          # (DIM, DIM)
    b_qk = np.concatenate([c[0], c[1]])                           # (2*DIM,)
    b_out = proj_b + c[3] + c[2] @ Wp                             # (DIM,)

    fcon = np.zeros((128, 24), np.float32)
    fcon[:, 0:8] = b_qk.reshape(OT, 128).T
    fcon[:, 8:12] = b_out.reshape(CT, 128).T

    def pmajor(w):
        # (DIM, W) -> (128, CT, W): partition p holds rows {p, 128+p, ...}
        return np.ascontiguousarray(
            w.reshape(CT, 128, w.shape[1]).transpose(1, 0, 2))

    mask_ones = bool(mask.all())
    wqk = pmajor(Wqkv[:, 0:2 * DIM])            # (128, CT, 1024)
    wqk_ord = np.concatenate([wqk[:, :, ot * 128:(ot + 1) * 128]
                              for ot in QK_ORDER], axis=2)
    com = {
        "wqk1": np.ascontiguousarray(wqk_ord[:, :, 0:256]).astype(bf16),
        "wqk2": np.ascontiguousarray(wqk_ord[:, :, 256:1024]).astype(bf16),
        "wvB": pmajor(Wqkv[:, 2 * DIM:3 * DIM]).astype(bf16),
        "wproj": pmajor(Wp).astype(bf16),
    }
    in_maps = []
    for b in range(B):
        m = dict(com)
        xp = pmajor(x[b].T.astype(np.float64))
        m["xA"] = np.ascontiguousarray(xp[:, :, 0:512]).astype(bf16)
        m["xB"] = np.ascontiguousarray(xp[:, :, 512:1024]).astype(bf16)
        fc = fcon.copy()
        if not mask_ones:
            mb = np.where(mask[b], 0.0, -1e30).astype(np.float32)
            fc[:, 12:20] = mb.reshape(NT, 128).T
        m["fcon"] = fc
        in_maps.append(m)
    return in_maps, mask_ones


LAST_EXEC_NS = None


def kernel(**inputs):
    global LAST_EXEC_NS
    in_maps, mask_ones = _prep(inputs)
    key = ("nc", mask_ones)
    if key not in _CACHE:
        _CACHE[key] = _build(mask_ones)
    nc = _CACHE[key]
    res = run_bass_kernel_spmd(nc, in_maps, core_ids=list(range(NCORES)))
    LAST_EXEC_NS = res.exec_time_ns
    outs = []
    for i in range(NCORES):
        ob = np.asarray(res.results[i]["out"], dtype=np.float32)  # (128,2,4,512)
        on = ob.transpose(2, 0, 1, 3).reshape(DIM, N)             # (o, n)
        outs.append(on.T.copy())
    return np.stack(outs).astype(np.float32)


# revision 18
# speedup vs baseline: 1.3199x; 1.0274x over previous
"""Trainium2 Bass kernel for nn_CP_Attention_Action (dense transformer block with
CP-factored low-rank corrections).

Data-parallel over batch B=8 -> one batch per NeuronCore, no collectives.

The CP branch is affine in its input, so it is folded into the dense weights on
the host (f64): W_qkv_eff = qkv_w.T + U.T @ F_f @ V.T blocks, with the q/k
biases applied at evacuation and the v bias folded through softmax (weights sum
to 1) into the proj bias. The device then runs a plain attention block:

  qkT (o,n) via stationary W tiles; v in natural (m,d) layout with a ones
  column so the softmax denominator falls out of attn@v; logits via 64x128
  row-group-paired matmuls (hl0/hl1 issued adjacently -> both stream on PE
  concurrently); exp on ScalarE in 1536-wide batches (all-ones mask -> uniform
  zero bias); denominator: copy+reciprocal on DVE + partition_broadcast on
  GpSimd; attn@v of pair p runs as PE filler during pair p+1; pair 3 uses an
  ncx-major st layout so its ncx0 attn@v half completes mid-pair and only the
  ncx1 half + proj remain in the tail. Host pre-arranges all inputs into
  partition-major layout for fat contiguous DMA descriptors on 4 queues.
"""

import os
import numpy as np
import ml_dtypes

from concourse import bacc
import concourse.mybir as mybir
import concourse.tile as tile
from concourse.bass_utils import run_bass_kernel_spmd

B, N, DIM = 8, 1024, 512
H, D = 8, 64
SCALE = D ** -0.5
NCORES = 8
NC_CH = 2          # n chunks of 512
NT = N // 128      # 8 key tiles
CT = DIM // 128    # 4 c-tiles
OT = 8             # q,k o-tiles
PAIRW = 2 * N      # free elems per key-tile slot in st (2 heads x 1024 q)
STW = NT * PAIRW   # st elems per pair (16384)
LPW = 1536         # activation batch width (3 PSUM banks)
F32 = mybir.dt.float32
BF = mybir.dt.bfloat16
AF = mybir.ActivationFunctionType
bf16 = ml_dtypes.bfloat16

_CACHE = {}


QK_ORDER = (0, 4, 1, 5, 2, 6, 3, 7)


def _stpos(pair, mt, hl, ncx):
    # ncx-major: the first half of each pair's stream only needs the first
    # n-half of q, so pair 0 can start before the second xT half lands
    return ncx * (NT * 1024) + mt * 1024 + hl * 512


def _slots(pair):
    # (mt, hl, ncx) in ascending stream-position order, hl adjacent
    out = []
    for ncx in range(NC_CH):
        for mt in range(NT):
            for hl in range(2):
                out.append((mt, hl, ncx))
    return out


def _build(mask_ones: bool):
    nc = bacc.Bacc()

    xA_e = nc.declare_dram_parameter("xA", [128, CT, 512], BF, isOutput=False)
    xB_e = nc.declare_dram_parameter("xB", [128, CT, 512], BF, isOutput=False)
    wqk1_e = nc.declare_dram_parameter("wqk1", [128, CT, 256], BF, isOutput=False)
    wqk2_e = nc.declare_dram_parameter("wqk2", [128, CT, 768], BF, isOutput=False)
    wvB_e = nc.declare_dram_parameter("wvB", [128, CT, DIM], BF, isOutput=False)
    wproj_e = nc.declare_dram_parameter("wproj", [128, CT, DIM], BF, isOutput=False)
    fcon_e = nc.declare_dram_parameter("fcon", [128, 24], F32, isOutput=False)
    out_e = nc.declare_dram_parameter("out", [128, NC_CH, CT, 512], BF, isOutput=True)

    # fcon layout (f32 columns): 0:8 qk bias per ot | 8:12 proj bias | 12:20 maskb
    with tile.TileContext(nc) as tc:
        with tc.tile_pool(name="consts", bufs=1) as consts, \
             tc.tile_pool(name="qkpool", bufs=1) as qkpool, \
             tc.tile_pool(name="stpool", bufs=2) as stpool, \
             tc.tile_pool(name="work", bufs=1) as work, \
             tc.tile_pool(name="dnpool", bufs=4) as dnpool, \
             tc.tile_pool(name="bcpool", bufs=2) as bcpool, \
             tc.tile_pool(name="outp", bufs=1) as outp, \
             tc.tile_pool(name="popool", bufs=1) as popool:

            # ---------- inputs: 4 DMA queues, partition-major layouts ----------
            warm = consts.tile([128, 512], BF)
            nc.vector.memset(warm[:], 0.0)
            scr = consts.tile([1, 16], BF)
            scr2 = consts.tile([1, 16], BF)
            nc.vector.memset(scr[:], 0.0)
            xh = [consts.tile([128, CT, 512], BF, name=f"xh{i}") for i in range(2)]
            fcon = consts.tile([128, 24], F32)
            wqk1 = consts.tile([128, CT, 256], BF)
            wqk2 = consts.tile([128, CT, 768], BF)
            nc.sync.dma_start(xh[0][:], xA_e[:, :, :])
            nc.sync.dma_start(xh[1][:], xB_e[:, :, :])
            nc.gpsimd.dma_start(wqk1[:], wqk1_e[:, :, :])
            nc.gpsimd.dma_start(wqk2[:], wqk2_e[:, :, :])
            nc.scalar.dma_start(fcon[:], fcon_e[:, :])
            wvB = consts.tile([128, CT, DIM], BF)
            nc.scalar.dma_start(wvB[:], wvB_e[:, :, :])
            wproj = consts.tile([128, CT, DIM], BF)
            nc.scalar.dma_start(wproj[:], wproj_e[:, :, :])

            def qkbias(ot):
                return fcon[:, ot:ot + 1]

            def pbias(ot):
                return fcon[:, 8 + ot:9 + ot]

            def maskb(mt):
                return fcon[:, 12 + mt:13 + mt]

            outT = outp.tile([128, CT, N], BF)
            v_sb = work.tile([128, NT, H, D + 1], BF)
            qk = qkpool.tile([128, OT, N], BF)
            nc.vector.memset(v_sb[:, :, :, D:D + 1], 1.0)
            # early exp-table load on a scratch tile (overlaps the DMA wait)
            nc.scalar.activation(scr2[:], scr[:], AF.Exp, bias=0.0, scale=1.0)

            # ---------- warm-up matmuls (HAM un-throttle during DMA wait) ----------
            wscr = consts.tile([1, 16], F32)
            with tc.tile_pool(name="wps", bufs=1, space="PSUM") as wps:
                wp = wps.tile([128, 512], F32)
                for i in range(4):
                    nc.tensor.matmul(wp[:, :], lhsT=warm[:, 0:128], rhs=warm[:, :],
                                     start=(i == 0), stop=(i == 3))
                # reader so later pools' bank reuse orders after the warm-up
                nc.vector.tensor_copy(wscr[0:1, :], wp[0:1, 0:16])

            with tc.tile_pool(name="lpp", bufs=2, space="PSUM") as lpp, \
                 tc.tile_pool(name="avp", bufs=2, space="PSUM") as avp:

                def qk_items(ot):
                    items = []
                    for ncx in range(NC_CH):
                        box = {}

                        def mk(ct, box=box, ncx=ncx, ot=ot):
                            def mm():
                                if "t" not in box:
                                    box["t"] = avp.tile([128, 512], F32, tag="av",
                                                        name=f"qp{ot}_{ncx}")
                                j = QK_ORDER.index(ot)
                                w = wqk1[:, ct, j * 128:(j + 1) * 128] if j < 2 \
                                    else wqk2[:, ct, (j - 2) * 128:(j - 1) * 128]
                                nc.tensor.matmul(
                                    box["t"][:, :], lhsT=w,
                                    rhs=xh[ncx][:, ct, :],
                                    start=(ct == 0), stop=(ct == CT - 1))
                            return mm

                        def ev(box=box, ncx=ncx, ot=ot):
                            nc.vector.tensor_scalar_add(
                                qk[:, ot, ncx * 512:(ncx + 1) * 512],
                                box["t"][:, :], qkbias(ot))
                        for ct in range(CT):
                            items.append(mk(ct))
                        items.append(ev)
                    return items

                # qk pair-0 weights first so pair 0 can start immediately
                for it in qk_items(0) + qk_items(4):
                    it()

                def v_items():
                    items = []
                    for mt in range(NT):
                        box = {}

                        def mk(ct, box=box, mt=mt):
                            def mm():
                                if "t" not in box:
                                    box["t"] = avp.tile([128, 512], F32, tag="av",
                                                        name=f"vp{mt}")
                                nc.tensor.matmul(
                                    box["t"][:, :],
                                    lhsT=xh[mt // 4][:, ct, (mt % 4) * 128:
                                                     (mt % 4) * 128 + 128],
                                    rhs=wvB[:, ct, :],
                                    start=(ct == 0), stop=(ct == CT - 1))
                            return mm

                        def ev(box=box, mt=mt):
                            nc.vector.tensor_copy(
                                v_sb[:, mt, :, 0:D],
                                box["t"][:, :].rearrange("p (h d) -> p h d", h=H))
                        for ct in range(CT):
                            items.append(mk(ct))
                        items.append(ev)
                    return items

                # ---------- attention ----------
                st_t = [None, None]

                def av_group(pair, hl, ncx, tail=False):
                    # attn@v accumulation + denominator chain for one head/ncx
                    items = []
                    stp = st_t[pair % 2]
                    h = 2 * pair + hl
                    box = {}

                    def mk(mt, box=box, hl=hl, ncx=ncx, h=h, pair=pair):
                        def mm():
                            if "t" not in box:
                                box["t"] = avp.tile([128, 512], F32, tag="av",
                                                    name=f"ap{h}_{ncx}")
                            p0 = _stpos(pair, mt, hl, ncx)
                            nc.tensor.matmul(
                                box["t"][0:D + 1, :],
                                lhsT=v_sb[:, mt, h, :],
                                rhs=stp[:, p0:p0 + 512],
                                start=(mt == 0), stop=(mt == NT - 1))
                        return mm

                    def chain(box=box, hl=hl, ncx=ncx, pair=pair, h=h, tail=tail):
                        # custom DVE ops drop the input base partition: copy the
                        # den row to partition 0 before the reciprocal
                        dnr = dnpool.tile([1, 512], F32, tag="dnr",
                                          name=f"dnr{h}_{ncx}")
                        if tail:   # ScalarE is idle after its last exp
                            nc.scalar.copy(dnr[0:1, :], box["t"][D:D + 1, :])
                        else:
                            nc.vector.tensor_copy(dnr[0:1, :], box["t"][D:D + 1, :])
                        dn = dnpool.tile([1, 512], F32, tag="dn",
                                         name=f"dn{h}_{ncx}")
                        nc.vector.reciprocal_approx_fast(dn[0:1, :], dnr[0:1, :])
                        bc = bcpool.tile([D, 512], F32, tag="bc",
                                         name=f"bc{h}_{ncx}")
                        nc.gpsimd.partition_broadcast(bc[:, :], dn[0:1, :])
                        nc.vector.tensor_mul(
                            outT[hl * 64:hl * 64 + 64, pair,
                                 ncx * 512:(ncx + 1) * 512],
                            box["t"][0:D, :], bc[:, :])
                    for mt in range(NT):
                        items.append(mk(mt))
                    items.append(chain)
                    return items

                po_sb = popool.tile([128, NC_CH, CT, 512], BF)

                def proj_items(ncx):
                    items = []
                    for ot in range(CT):
                        box = {}

                        def mk(kt, box=box, ot=ot, ncx=ncx):
                            def mm():
                                if "t" not in box:
                                    box["t"] = lpp.tile([128, LPW], F32, tag="lp",
                                                        name=f"pp{ot % 2}")
                                nc.tensor.matmul(
                                    box["t"][:, 0:512],
                                    lhsT=wproj[:, kt, ot * 128:(ot + 1) * 128],
                                    rhs=outT[:, kt, ncx * 512:(ncx + 1) * 512],
                                    start=(kt == 0), stop=(kt == CT - 1))
                            return mm

                        def ev(box=box, ot=ot, ncx=ncx):
                            if ncx == 1:   # tail: ScalarE is idle, DVE is not
                                nc.scalar.add(po_sb[:, ncx, ot, :],
                                              box["t"][:, 0:512], pbias(ot))
                            else:
                                nc.vector.tensor_scalar_add(
                                    po_sb[:, ncx, ot, :], box["t"][:, 0:512],
                                    pbias(ot))
                        for kt in range(CT):
                            items.append(mk(kt))
                        items.append(ev)

                    def dma(ncx=ncx):
                        deng = nc.sync if ncx == 0 else nc.gpsimd
                        deng.dma_start(out_e[:, ncx, :, :], po_sb[:, ncx, :, :])
                    items.append(dma)
                    return items

                filler = []
                fi = 0

                def consume(k):
                    nonlocal fi
                    e = min(fi + k, len(filler))
                    while fi < e:
                        filler[fi]()
                        fi += 1

                for pair in range(4):
                    stp = stpool.tile([128, STW], BF, tag="st", name=f"st{pair % 2}")
                    st_t[pair % 2] = stp
                    if pair == 0:
                        filler = v_items()
                        for ot in (1, 5, 2, 6):
                            filler += qk_items(ot)
                        fi = 0
                    elif pair == 1:
                        # av(0) (set at end of pair 0) + held-back qk
                        filler = filler[fi:] + qk_items(3) + qk_items(7)
                        fi = 0
                    elif pair == 3:
                        # av(2) plus the ncx0 half of av(3) (ncx-major st layout
                        # means its chunks complete in the first half of pair 3)
                        rem = filler[fi:]
                        filler = rem + av_group(3, 0, 0) + av_group(3, 1, 0)
                        fi = 0
                    npts = len(_slots(pair)) // 2
                    per = max(1, (len(filler) - fi + npts - 1) // npts)

                    lpt = {}
                    acted = 0
                    for si, (mt, hl, ncx) in enumerate(_slots(pair)):
                        pos = _stpos(pair, mt, hl, ncx)
                        ti = pos // LPW
                        off = pos - ti * LPW
                        if ti not in lpt:
                            lpt[ti] = lpp.tile([128, LPW], F32, tag="lp",
                                               name=f"lp{pair}_{ti % 2}")
                        pb = hl * 64
                        nc.tensor.matmul(
                            lpt[ti][:, off:off + 512],
                            lhsT=qk[pb:pb + 64, 4 + pair, mt * 128:(mt + 1) * 128],
                            rhs=qk[pb:pb + 64, pair, ncx * 512:(ncx + 1) * 512],
                            start=True, stop=True, tile_position=(pb, 0))
                        if not mask_ones:
                            nc.scalar.activation(
                                stp[:, pos:pos + 512], lpt[ti][:, off:off + 512],
                                AF.Exp, bias=maskb(mt), scale=SCALE)
                            if off + 512 == LPW or pos + 512 == STW:
                                del lpt[ti]
                        else:
                            while acted * LPW + LPW <= pos + 512 or \
                                    pos + 512 == STW:
                                w = min(LPW, STW - acted * LPW)
                                nc.scalar.activation(
                                    stp[:, acted * LPW:acted * LPW + w],
                                    lpt[acted][:, 0:w], AF.Exp,
                                    bias=0.0, scale=SCALE)
                                del lpt[acted]
                                acted += 1
                                if acted * LPW >= STW:
                                    break
                        if si % 2 == 1:
                            consume(per)   # only between hl pairs (PE row pairing)

                    consume(len(filler))
                    if pair < 3:
                        filler = []
                        for ncx in range(NC_CH):
                            for hl in range(2):
                                filler += av_group(pair, hl, ncx)
                        fi = 0

                # ---------- tail: av(3) ncx1 + proj ----------
                filler = av_group(3, 0, 1, tail=True) + av_group(3, 1, 1, tail=True)
                fi = 0
                tail = proj_items(0)
                ti2 = 0
                for it in filler:
                    it()
                    # interleave proj(ncx0) (independent of av(3) ncx1)
                    if ti2 < len(tail):
                        tail[ti2]()
                        ti2 += 1
                while ti2 < len(tail):
                    tail[ti2]()
                    ti2 += 1
                for it in proj_items(1):
                    it()

    nc.compile()
    return nc


def _prep(inputs):
    x = np.asarray(inputs["x"])
    mask = np.asarray(inputs["mask"])
    qkv_w = np.asarray(inputs["qkv_w"], np.float64)
    CP_U_w = np.asarray(inputs["CP_U_w"], np.float64)
    CP_U_b = np.asarray(inputs["CP_U_b"], np.float64)
    CP_V_w = np.asarray(inputs["CP_V_w"], np.float64)
    CP_V_b = np.asarray(inputs["CP_V_b"], np.float64)
    CP_C = np.asarray(inputs["CP_C"], np.float64)
    CP_att = np.asarray(inputs["CP_attention"], np.float64)
    proj_w = np.asarray(inputs["proj_w"], np.float64)
    proj_b = np.asarray(inputs["proj_b"], np.float64)

    # fold the CP branch (affine in its input) into the dense weights
    F = np.einsum('ijr,rf->fij', CP_C, CP_att)          # (4, R, R)
    UT = CP_U_w.T                                        # (DIM, R)
    VT = CP_V_w.T                                        # (R, DIM)
    A = np.stack([UT @ F[f] @ VT for f in range(4)])     # (4, DIM, DIM)
    c = np.stack([CP_U_b @ F[f] @ VT + CP_V_b for f in range(4)])  # (4, DIM)

    Wqkv = qkv_w.T + np.concatenate([A[0], A[1], A[2]], axis=1)   # (DIM, 3*DIM)
    Wp = proj_w.T + A[3]                                          # (DIM, DIM)
    b_qk = np.concatenate([c[0], c[1]])                           # (2*DIM,)
    b_out = proj_b + c[3] + c[2] @ Wp                             # (DIM,)

    fcon = np.zeros((128, 24), np.float32)
    fcon[:, 0:8] = b_qk.reshape(OT, 128).T
    fcon[:, 8:12] = b_out.reshape(CT, 128).T

    def pmajor(w):
        # (DIM, W) -> (128, CT, W): partition p holds rows {p, 128+p, ...}
        return np.ascontiguousarray(
            w.reshape(CT, 128, w.shape[1]).transpose(1, 0, 2))

    mask_ones = bool(mask.all())
    wqk = pmajor(Wqkv[:, 0:2 * DIM])            # (128, CT, 1024)
    wqk_ord = np.concatenate([wqk[:, :, ot * 128:(ot + 1) * 128]
                              for ot in QK_ORDER], axis=2)
    com = {
        "wqk1": np.ascontiguousarray(wqk_ord[:, :, 0:256]).astype(bf16),
        "wqk2": np.ascontiguousarray(wqk_ord[:, :, 256:1024]).astype(bf16),
        "wvB": pmajor(Wqkv[:, 2 * DIM:3 * DIM]).astype(bf16),
        "wproj": pmajor(Wp).astype(bf16),
    }
    in_maps = []
    for b in range(B):
        m = dict(com)
        xp = pmajor(x[b].T.astype(np.float64))
        m["xA"] = np.ascontiguousarray(xp[:, :, 0:512]).astype(bf16)
        m["xB"] = np.ascontiguousarray(xp[:, :, 512:1024]).astype(bf16)
        fc = fcon.copy()
        if not mask_ones:
            mb = np.where(mask[b], 0.0, -1e30).astype(np.float32)
            fc[:, 12:20] = mb.reshape(NT, 128).T
        m["fcon"] = fc
        in_maps.append(m)
    return in_maps, mask_ones


LAST_EXEC_NS = None


def kernel(**inputs):
    global LAST_EXEC_NS
    in_maps, mask_ones = _prep(inputs)
    key = ("nc", mask_ones)
    if key not in _CACHE:
        _CACHE[key] = _build(mask_ones)
    nc = _CACHE[key]
    res = run_bass_kernel_spmd(nc, in_maps, core_ids=list(range(NCORES)))
    LAST_EXEC_NS = res.exec_time_ns
    outs = []
    for i in range(NCORES):
        ob = np.asarray(res.results[i]["out"], dtype=np.float32)  # (128,2,4,512)
        on = ob.transpose(2, 0, 1, 3).reshape(DIM, N)             # (o, n)
        outs.append(on.T.copy())
    return np.stack(outs).astype(np.float32)
